# revision 14
# baseline (speedup 1.0000x reference)
"""Trainium2 Bass kernel for nn_DynamicMaxSimilarity: scan-based dual-sweep DP.

Full inputs a,b: [512, 16, 256] f32.
  an = l2norm(tanh(a)) rows; bn likewise
  sim[a,b,i,j] = dot(an[a,i], bn[b,j]);  out[a,b] = DTW-like max-avg DP.

Sharding: 8 cores as 4 a-chunks (128) x 2 b-chunks (256). Per-core block
[128 a, 256 b]; pairs live as [128 partitions (a), 256 free (b)].

DP in the scaled domain u[i,j] = si[i,j]*max(i,j):
  step k: diag (k,k); row sweep cells (k, j>k); col sweep cells (i>k, k).
  Row sweep, per cell j: u = max(t[j], u_left) + lc, with
    t[j] = max(u_prev[j-1], u_prev[j]*(j-1)/j)   (prep, bulk)
  done by ONE tensor_tensor_scan (op0=max, op1=add) over free axis
  (pair-major, slots [inject, data...]): the inject slot (d0=diag+BIG,
  d1=-BIG) resets and seeds the per-pair chain. Col sweep symmetric.
Validated exactly vs the reference recurrence in fp64.
"""

import numpy as np

import concourse.bass as bass
from concourse import bacc
import concourse.mybir as mybir
from concourse.tile import TileContext
from concourse import bass_utils

NA, NB, T, D = 512, 512, 16, 256
ACH, BCH = 128, 256
P = 128
F = BCH              # pairs per partition
KH = D // 128
DT = mybir.dt.float32
F32R = mybir.dt.float32r
ALU = mybir.AluOpType
ACTF = mybir.ActivationFunctionType
BIG = 64.0

_last_results = None


def _normalize_block(nc, pool, wp, x_sb, nt, name):
    nc.scalar.activation(x_sb[:, :, :], x_sb[:, :, :], ACTF.Tanh)
    ssq = pool.tile([P, nt], DT, name=f"{name}_ssq")
    for i in range(nt):
        scr = wp.tile([P, D], DT, name=f"{name}_sq{i}", tag="sq_scr")
        nc.scalar.activation(scr[:, :], x_sb[:, i, :], ACTF.Square,
                             accum_out=ssq[:, i:i + 1])
    nrm = pool.tile([P, nt], DT, name=f"{name}_nrm")
    nc.scalar.activation(nrm[:, :], ssq[:, :], ACTF.Sqrt)
    rinv = pool.tile([P, nt], DT, name=f"{name}_rinv")
    nc.vector.reciprocal(rinv[:, :], nrm[:, :])
    for i in range(nt):
        nc.vector.tensor_scalar_mul(x_sb[:, i, :], x_sb[:, i, :],
                                    rinv[:, i:i + 1])
    return x_sb


def build_program():
    nc = bacc.Bacc("TRN2", target_bir_lowering=False, debug=False)

    a_d = nc.dram_tensor("a_c", [ACH, T, D], DT, kind="ExternalInput")
    b_d = nc.dram_tensor("b_c", [BCH, T, D], DT, kind="ExternalInput")
    ident_d = nc.dram_tensor("ident", [128, 128], DT, kind="ExternalInput")
    rbig_d = nc.dram_tensor("rbig", [P, 17 * F], DT, kind="ExternalInput")
    out_d = nc.dram_tensor("out", [ACH, BCH], DT, kind="ExternalOutput")

    with TileContext(nc) as tc:
        with (
            tc.tile_pool(name="mp", bufs=1) as mp,
            tc.tile_pool(name="wp", bufs=2) as wp,
        ):
            ident = mp.tile([128, 128], DT)
            nc.sync.dma_start(ident[:, :], ident_d.ap())
            # rbig[m*F + p] = (m-1)/m  (slot-major, constant across pairs)
            rbig = mp.tile([P, 17, F], DT)
            nc.sync.dma_start(rbig[:, :, :],
                              rbig_d.ap().rearrange("p (m f) -> p m f", f=F))

            # ---- load + normalize (in place) ----
            a_sb = mp.tile([P, T, D], DT, tag="sh_lrow")
            for q in range(2):
                nc.sync.dma_start(a_sb[:, q * 8:(q + 1) * 8, :],
                                  a_d.ap()[:, q * 8:(q + 1) * 8, :])
            an = _normalize_block(nc, mp, wp, a_sb, T, "a")

            b_sb = [mp.tile([P, T, D], DT, name=f"b_sb{h}", tag=f"sh_b{h}")
                    for h in range(2)]
            for h in range(2):
                for q in range(2):
                    nc.sync.dma_start(
                        b_sb[h][:, q * 8:(q + 1) * 8, :],
                        b_d.ap()[h * 128:(h + 1) * 128, q * 8:(q + 1) * 8, :])
            bn = [_normalize_block(nc, mp, wp, b_sb[h], T, f"b{h}")
                  for h in range(2)]

            # ---- transposes into matmul layout ----
            anT = mp.tile([P, T, 2, 128], F32R)
            bnT = [mp.tile([P, T, BCH], F32R, name=f"bnT{kh}") for kh in range(KH)]
            with tc.tile_pool(name="tpp", bufs=4, space="PSUM") as tpp:
                # a-side: 4 transposes (2 i x 2 kh) per PSUM bank, 1 evict each
                for i0 in range(0, T, 2):
                    ps = tpp.tile([128, 512], DT, name=f"tpa{i0}", tag="tpa")
                    for q, (i, kh) in enumerate(
                            (i0 + di, kh) for di in range(2) for kh in range(KH)):
                        nc.tensor.transpose(ps[:, q * 128:(q + 1) * 128],
                                            an[:, i, kh * 128:(kh + 1) * 128],
                                            ident[:, :])
                    nc.scalar.activation(
                        anT[:, i0:i0 + 2, :, :],
                        ps[:, :].rearrange("p (i k a) -> p i k a", i=2, k=2),
                        ACTF.Copy)
                # b-side: 2 transposes (2 bh) per (j, kh), 1 evict each
                for j in range(T):
                    for kh in range(KH):
                        ps = tpp.tile([128, 256], DT, name=f"tpb{j}{kh}",
                                      tag="tpb")
                        for bh in range(2):
                            nc.tensor.transpose(
                                ps[:, bh * 128:(bh + 1) * 128],
                                bn[bh][:, j, kh * 128:(kh + 1) * 128],
                                ident[:, :])
                        nc.scalar.activation(bnT[kh][:, j, :], ps[:, :],
                                             ACTF.Copy)

            # ---- DP tiles (flat [P, 17*F]; triangular strided views) ----
            # row sweep ping-pong rA/rB; col sweep cA/cB; d1 tiles lrow/lcol.
            # a_sb/b_sb memory is reused for lrow/cB/lcol via tag sharing.
            rA = mp.tile([P, 17 * F], DT)
            rB = mp.tile([P, 17 * F], DT)
            cA = mp.tile([P, 17 * F], DT)
            cB = mp.tile([P, 17 * F], DT, tag="sh_b0")
            lrow = mp.tile([P, 17 * F], DT, tag="sh_lrow")
            lcol = mp.tile([P, 17 * F], DT, tag="sh_b1")
            ud = [mp.tile([P, F], DT, name=f"ud{x}") for x in range(2)]

            def tri(tile, k):
                """[P, slots, pairs] view with pair-stride S=17-k: slot s of
                pair p at flat offset p*S + s. Returns AP [P, S, F]."""
                S = 17 - k
                return tile[:, 0:S * F].rearrange("p (f s) -> p s f", s=S)

            row_prev = row_cur = col_prev = col_cur = None

            # ---- fused matmul + dual-sweep DP ----
            with tc.tile_pool(name="pp", bufs=2, space="PSUM") as pp:
                def mm_planes(psv, i0, j0, np_):
                    """matmul lc planes (frame i0; b-frames j0..j0+np_) into
                    psv [128, np_*F]."""
                    for q0 in range(0, np_, 2):
                        w = min(2, np_ - q0)
                        for kh in range(KH):
                            nc.tensor.matmul(
                                psv[:, q0 * F:(q0 + w) * F],
                                anT[:, i0, kh, :],
                                bnT[kh][:, j0 + q0:j0 + q0 + w, :],
                                start=(kh == 0), stop=(kh == KH - 1),
                            )

                for k in range(1, T + 1):
                    S = 17 - k
                    udp, udc = ud[(k - 1) % 2], ud[k % 2]

                    # --- produce lc L-border for step k ---
                    # diag+row piece: frame k-1 x b-frames k-1..15
                    nrow = 17 - k   # includes diag plane
                    row_chunks = []
                    for c0 in range(0, nrow, 4):
                        w = min(4, nrow - c0)
                        psv = pp.tile([128, 4 * F], DT,
                                      name=f"psr{k}_{c0}", tag="locr")
                        mm_planes(psv, k - 1, (k - 1) + c0, w)
                        row_chunks.append((c0, w, psv))
                    # col piece: frames k..15 x b-frame k-1
                    ncol = 16 - k
                    col_chunks = []
                    for c0 in range(0, ncol, 4):
                        w = min(4, ncol - c0)
                        psv = pp.tile([128, 4 * F], DT,
                                      name=f"psc{k}_{c0}", tag="locc")
                        for q in range(w):
                            for kh in range(KH):
                                nc.tensor.matmul(
                                    psv[:, q * F:(q + 1) * F],
                                    anT[:, k + c0 + q, kh, :],
                                    bnT[kh][:, k - 1:k, :],
                                    start=(kh == 0), stop=(kh == KH - 1),
                                )
                        col_chunks.append((c0, w, psv))

                    # --- diag cell (k,k) ---
                    lc_diag = row_chunks[0][2][:, 0:F]
                    if k == 1:
                        nc.vector.tensor_scalar_add(udc[:, :], lc_diag, 0.0)
                    else:
                        m1 = wp.tile([P, F], DT, name=f"m1_{k}", tag="m1")
                        nc.vector.tensor_tensor(
                            m1[:, :], tri(row_prev, k - 1)[:, 1, :],
                            tri(col_prev, k - 1)[:, 1, :], ALU.max)
                        nc.vector.scalar_tensor_tensor(
                            m1[:, :], m1[:, :], float((k - 1) / k), udp[:, :],
                            ALU.mult, ALU.max)
                        nc.vector.tensor_tensor(udc[:, :], m1[:, :], lc_diag,
                                                ALU.add)

                    if k == T:
                        break

                    # --- sweeps ---
                    row_cur = rA if (k % 2) else rB
                    col_cur = cA if (k % 2) else cB
                    tr_cur, tc_cur = tri(row_cur, k), tri(col_cur, k)
                    tl_row, tl_col = tri(lrow, k), tri(lcol, k)

                    # injects: d0 = diag + BIG (ACT), d1 = -BIG (gpsimd)
                    nc.scalar.activation(tr_cur[:, 0, :], udc[:, :], ACTF.Copy,
                                         bias=BIG)
                    nc.scalar.activation(tc_cur[:, 0, :], udc[:, :], ACTF.Copy,
                                         bias=BIG)
                    nc.gpsimd.memset(tl_row[:, 0, :], -BIG)
                    nc.gpsimd.memset(tl_col[:, 0, :], -BIG)

                    # evictions into d1 data slots
                    for (c0, w, psv) in row_chunks:
                        if c0 == 0 and w == 1:
                            continue   # only-diag chunk
                        # chunk covers b-frames (k-1)+c0 .. +w-1; frame p is
                        # cell j=p+1 at d1 slot j-k = p-k+1; skip diag (c0=0).
                        q0 = 1 if c0 == 0 else 0     # local first plane
                        s0 = c0 + q0                 # dest slot of that plane
                        nc.scalar.activation(
                            tl_row[:, s0:c0 + w, :],
                            psv[:, q0 * F:w * F].rearrange(
                                "p (n f) -> p n f", f=F),
                            ACTF.Copy)
                    for (c0, w, psv) in col_chunks:
                        nc.scalar.activation(
                            tl_col[:, 1 + c0:1 + c0 + w, :],
                            psv[:, 0:w * F].rearrange("p (n f) -> p n f", f=F),
                            ACTF.Copy)

                    # prep t data slots
                    if k == 1:
                        nc.gpsimd.memset(tr_cur[:, 1:S, :], 0.0)
                        nc.gpsimd.memset(tc_cur[:, 1:S, :], 0.0)
                    else:
                        trp, tcp = tri(row_prev, k - 1), tri(col_prev, k - 1)
                        # t[j] = max(u_prev[j-1], u_prev[j] * (j-1)/j)
                        # prev data: cell j' at slot j'-(k-1); j=k+1..16:
                        #   u_prev[j]   -> prev slots 2..S_prev-1
                        #   u_prev[j-1] -> prev slots 1..S_prev-2
                        nc.vector.tensor_tensor(
                            tr_cur[:, 1:S, :], trp[:, 2:S + 1, :],
                            rbig[:, k + 1:17, :], ALU.mult)
                        nc.vector.tensor_tensor(
                            tr_cur[:, 1:S, :], tr_cur[:, 1:S, :],
                            trp[:, 1:S, :], ALU.max)
                        nc.vector.tensor_tensor(
                            tc_cur[:, 1:S, :], tcp[:, 2:S + 1, :],
                            rbig[:, k + 1:17, :], ALU.mult)
                        nc.vector.tensor_tensor(
                            tc_cur[:, 1:S, :], tc_cur[:, 1:S, :],
                            tcp[:, 1:S, :], ALU.max)

                    # scans (in place on the t tiles)
                    nc.vector.tensor_tensor_scan(
                        row_cur[:, 0:S * F], row_cur[:, 0:S * F],
                        lrow[:, 0:S * F], 0.0, ALU.max, ALU.add)
                    nc.vector.tensor_tensor_scan(
                        col_cur[:, 0:S * F], col_cur[:, 0:S * F],
                        lcol[:, 0:S * F], 0.0, ALU.max, ALU.add)

                    row_prev, col_prev = row_cur, col_cur

            out_sb = mp.tile([P, F], DT)
            nc.vector.tensor_scalar_mul(out_sb[:, :], ud[T % 2][:, :], 1.0 / T)
            nc.sync.dma_start(out_d.ap(), out_sb[:, :])

    nc.compile()
    return nc


def _consts():
    ident = np.eye(128, dtype=np.float32)
    rb = np.zeros((17, F), dtype=np.float32)
    for m in range(1, 17):
        rb[m, :] = (m - 1) / m
    rbig = np.broadcast_to(rb.reshape(1, 17 * F), (P, 17 * F)).copy()
    return ident, rbig


def kernel(a: np.ndarray, b: np.ndarray) -> np.ndarray:
    a = np.ascontiguousarray(a, dtype=np.float32)
    b = np.ascontiguousarray(b, dtype=np.float32)
    assert a.shape == (NA, T, D) and b.shape == (NB, T, D)

    nc = build_program()
    ident, rbig = _consts()

    in_maps = []
    for core in range(8):
        ca, cb = core // 2, core % 2
        in_maps.append({
            "a_c": a[ca * ACH:(ca + 1) * ACH],
            "b_c": b[cb * BCH:(cb + 1) * BCH],
            "ident": ident,
            "rbig": rbig,
        })

    res = bass_utils.run_bass_kernel_spmd(nc, in_maps, core_ids=list(range(8)))
    global _last_results
    _last_results = res

    out = np.zeros((NA, NB), dtype=np.float32)
    for core in range(8):
        ca, cb = core // 2, core % 2
        out[ca * ACH:(ca + 1) * ACH, cb * BCH:(cb + 1) * BCH] = \
            res.results[core]["out"]
    return out


# revision 15
# speedup vs baseline: 1.0508x; 1.0508x over previous
"""Trainium2 Bass kernel for nn_DynamicMaxSimilarity — scan-based dual-sweep DP.

Full inputs a,b: [512, 16, 256] f32.
  an = l2norm(tanh(a)) rows; bn likewise
  sim[a,b,i,j] = dot(an[a,i], bn[b,j]);  out[a,b] = DTW-like max-avg DP.

Sharding: 8 cores as 4 a-chunks (128) x 2 b-chunks (256). Per-core block
[128 a, 256 b]; pairs live as [128 partitions (a), 256 free (b)].

DP in the scaled domain u[i,j] = si[i,j]*max(i,j):
  step k: diag (k,k); row sweep cells (k, j>k); col sweep cells (i>k, k).
  Row sweep, per cell j: u = max(t[j], u_left) + lc, with
    t[j] = max(u_prev[j-1], u_prev[j]*(j-1)/j)   (prep, bulk)
  done by ONE tensor_tensor_scan (op0=max, op1=add) over free axis
  (pair-major, slots [inject, data...]): the inject slot (d0=diag+BIG,
  d1=-BIG) resets and seeds the per-pair chain. Col sweep symmetric.
Validated exactly vs the reference recurrence in fp64.
"""

import numpy as np

import concourse.bass as bass
from concourse import bacc
import concourse.mybir as mybir
from concourse.tile import TileContext
from concourse import bass_utils

NA, NB, T, D = 512, 512, 16, 256
ACH, BCH = 128, 256
P = 128
F = BCH              # pairs per partition
KH = D // 128
DT = mybir.dt.float32
F32R = mybir.dt.float32r
ALU = mybir.AluOpType
ACTF = mybir.ActivationFunctionType
BIG = 64.0

_last_results = None


def _normalize_block(nc, pool, wp, x_sb, nt, name):
    nc.scalar.activation(x_sb[:, :, :], x_sb[:, :, :], ACTF.Tanh)
    ssq = pool.tile([P, nt], DT, name=f"{name}_ssq")
    sq = wp.tile([P, nt, D], DT, name=f"{name}_sq", tag="sq_scr")
    nc.vector.tensor_tensor(sq[:, :, :], x_sb[:, :, :], x_sb[:, :, :], ALU.mult)
    nc.vector.tensor_reduce(ssq[:, :], sq[:, :, :], mybir.AxisListType.X,
                            ALU.add)
    nrm = pool.tile([P, nt], DT, name=f"{name}_nrm")
    nc.scalar.activation(nrm[:, :], ssq[:, :], ACTF.Sqrt)
    rinv = pool.tile([P, nt], DT, name=f"{name}_rinv")
    nc.vector.reciprocal(rinv[:, :], nrm[:, :])
    for i in range(nt):
        nc.vector.tensor_scalar_mul(x_sb[:, i, :], x_sb[:, i, :],
                                    rinv[:, i:i + 1])
    return x_sb


def build_program():
    nc = bacc.Bacc("TRN2", target_bir_lowering=False, debug=False)

    a_d = nc.dram_tensor("a_c", [ACH, T, D], DT, kind="ExternalInput")
    b_d = nc.dram_tensor("b_c", [BCH, T, D], DT, kind="ExternalInput")
    ident_d = nc.dram_tensor("ident", [128, 128], DT, kind="ExternalInput")
    rbig_d = nc.dram_tensor("rbig", [P, 17 * F], DT, kind="ExternalInput")
    out_d = nc.dram_tensor("out", [ACH, BCH], DT, kind="ExternalOutput")

    with TileContext(nc) as tc:
        with (
            tc.tile_pool(name="mp", bufs=1) as mp,
            tc.tile_pool(name="wp", bufs=2) as wp,
        ):
            ident = mp.tile([128, 128], DT)
            nc.sync.dma_start(ident[:, :], ident_d.ap())
            # rbig[m*F + p] = (m-1)/m  (slot-major, constant across pairs)
            rbig = mp.tile([P, 17, F], DT)
            nc.sync.dma_start(rbig[:, :, :],
                              rbig_d.ap().rearrange("p (m f) -> p m f", f=F))

            # ---- load + normalize (in place) ----
            a_sb = mp.tile([P, T, D], DT, tag="sh_lrow")
            for q in range(2):
                nc.sync.dma_start(a_sb[:, q * 8:(q + 1) * 8, :],
                                  a_d.ap()[:, q * 8:(q + 1) * 8, :])
            an = _normalize_block(nc, mp, wp, a_sb, T, "a")

            b_sb = [mp.tile([P, T, D], DT, name=f"b_sb{h}", tag=f"sh_b{h}")
                    for h in range(2)]
            for h in range(2):
                for q in range(2):
                    nc.sync.dma_start(
                        b_sb[h][:, q * 8:(q + 1) * 8, :],
                        b_d.ap()[h * 128:(h + 1) * 128, q * 8:(q + 1) * 8, :])
            bn = [_normalize_block(nc, mp, wp, b_sb[h], T, f"b{h}")
                  for h in range(2)]

            # ---- transposes into matmul layout ----
            anT = mp.tile([P, T, 2, 128], F32R)
            bnT = [mp.tile([P, T, BCH], F32R, name=f"bnT{kh}") for kh in range(KH)]
            with tc.tile_pool(name="tpp", bufs=4, space="PSUM") as tpp:
                # a-side: 4 transposes (2 i x 2 kh) per PSUM bank, 1 evict each
                for i0 in range(0, T, 2):
                    ps = tpp.tile([128, 512], DT, name=f"tpa{i0}", tag="tpa")
                    for q, (i, kh) in enumerate(
                            (i0 + di, kh) for di in range(2) for kh in range(KH)):
                        nc.tensor.transpose(ps[:, q * 128:(q + 1) * 128],
                                            an[:, i, kh * 128:(kh + 1) * 128],
                                            ident[:, :])
                    nc.scalar.activation(
                        anT[:, i0:i0 + 2, :, :],
                        ps[:, :].rearrange("p (i k a) -> p i k a", i=2, k=2),
                        ACTF.Copy)
                # b-side: 2 transposes (2 bh) per (j, kh), 1 evict each
                for j in range(T):
                    for kh in range(KH):
                        ps = tpp.tile([128, 256], DT, name=f"tpb{j}{kh}",
                                      tag="tpb")
                        for bh in range(2):
                            nc.tensor.transpose(
                                ps[:, bh * 128:(bh + 1) * 128],
                                bn[bh][:, j, kh * 128:(kh + 1) * 128],
                                ident[:, :])
                        if j % 2 == 0:
                            nc.scalar.activation(bnT[kh][:, j, :], ps[:, :],
                                                 ACTF.Copy)
                        else:
                            nc.vector.tensor_copy(bnT[kh][:, j, :], ps[:, :])

            # ---- DP tiles (flat [P, 17*F]; triangular strided views) ----
            # row sweep ping-pong rA/rB; col sweep cA/cB; d1 tiles lrow/lcol.
            # a_sb/b_sb memory is reused for lrow/cB/lcol via tag sharing.
            rA = mp.tile([P, 17 * F], DT)
            rB = mp.tile([P, 17 * F], DT)
            cA = mp.tile([P, 17 * F], DT)
            cB = mp.tile([P, 17 * F], DT, tag="sh_b0")
            lrow = mp.tile([P, 17 * F], DT, tag="sh_lrow")
            lcol = mp.tile([P, 17 * F], DT, tag="sh_b1")
            ud = [mp.tile([P, F], DT, name=f"ud{x}") for x in range(2)]

            def tri(tile, k):
                """[P, slots, pairs] view with pair-stride S=17-k: slot s of
                pair p at flat offset p*S + s. Returns AP [P, S, F]."""
                S = 17 - k
                return tile[:, 0:S * F].rearrange("p (f s) -> p s f", s=S)

            row_prev = row_cur = col_prev = col_cur = None

            # ---- fused matmul + dual-sweep DP ----
            with tc.tile_pool(name="pp", bufs=2, space="PSUM") as pp:
                def mm_planes(psv, i0, j0, np_):
                    """matmul lc planes (frame i0; b-frames j0..j0+np_) into
                    psv [128, np_*F]."""
                    for q0 in range(0, np_, 2):
                        w = min(2, np_ - q0)
                        for kh in range(KH):
                            nc.tensor.matmul(
                                psv[:, q0 * F:(q0 + w) * F],
                                anT[:, i0, kh, :],
                                bnT[kh][:, j0 + q0:j0 + q0 + w, :],
                                start=(kh == 0), stop=(kh == KH - 1),
                            )

                for k in range(1, T + 1):
                    S = 17 - k
                    udp, udc = ud[(k - 1) % 2], ud[k % 2]

                    # --- produce lc L-border for step k ---
                    # diag+row piece: frame k-1 x b-frames k-1..15
                    nrow = 17 - k   # includes diag plane
                    row_chunks = []
                    for c0 in range(0, nrow, 4):
                        w = min(4, nrow - c0)
                        psv = pp.tile([128, 4 * F], DT,
                                      name=f"psr{k}_{c0}", tag="locr")
                        mm_planes(psv, k - 1, (k - 1) + c0, w)
                        row_chunks.append((c0, w, psv))
                    # col piece: frames k..15 x b-frame k-1
                    ncol = 16 - k
                    col_chunks = []
                    for c0 in range(0, ncol, 4):
                        w = min(4, ncol - c0)
                        psv = pp.tile([128, 4 * F], DT,
                                      name=f"psc{k}_{c0}", tag="locc")
                        for q in range(w):
                            for kh in range(KH):
                                nc.tensor.matmul(
                                    psv[:, q * F:(q + 1) * F],
                                    anT[:, k + c0 + q, kh, :],
                                    bnT[kh][:, k - 1:k, :],
                                    start=(kh == 0), stop=(kh == KH - 1),
                                )
                        col_chunks.append((c0, w, psv))

                    # --- diag cell (k,k) ---
                    lc_diag = row_chunks[0][2][:, 0:F]
                    if k == 1:
                        nc.vector.tensor_scalar_add(udc[:, :], lc_diag, 0.0)
                    else:
                        m1 = wp.tile([P, F], DT, name=f"m1_{k}", tag="m1")
                        nc.vector.tensor_tensor(
                            m1[:, :], tri(row_prev, k - 1)[:, 1, :],
                            tri(col_prev, k - 1)[:, 1, :], ALU.max)
                        nc.vector.scalar_tensor_tensor(
                            m1[:, :], m1[:, :], float((k - 1) / k), udp[:, :],
                            ALU.mult, ALU.max)
                        nc.vector.tensor_tensor(udc[:, :], m1[:, :], lc_diag,
                                                ALU.add)

                    if k == T:
                        break

                    # --- sweeps ---
                    row_cur = rA if (k % 2) else rB
                    col_cur = cA if (k % 2) else cB
                    tr_cur, tc_cur = tri(row_cur, k), tri(col_cur, k)
                    tl_row, tl_col = tri(lrow, k), tri(lcol, k)

                    # injects: d0 = diag + BIG (ACT), d1 = -BIG (gpsimd)
                    nc.scalar.activation(tr_cur[:, 0, :], udc[:, :], ACTF.Copy,
                                         bias=BIG)
                    nc.scalar.activation(tc_cur[:, 0, :], udc[:, :], ACTF.Copy,
                                         bias=BIG)
                    nc.gpsimd.memset(tl_row[:, 0, :], -BIG)
                    nc.gpsimd.memset(tl_col[:, 0, :], -BIG)

                    # evictions into d1 data slots
                    for (c0, w, psv) in row_chunks:
                        if c0 == 0 and w == 1:
                            continue   # only-diag chunk
                        # chunk covers b-frames (k-1)+c0 .. +w-1; frame p is
                        # cell j=p+1 at d1 slot j-k = p-k+1; skip diag (c0=0).
                        q0 = 1 if c0 == 0 else 0     # local first plane
                        s0 = c0 + q0                 # dest slot of that plane
                        nc.scalar.activation(
                            tl_row[:, s0:c0 + w, :],
                            psv[:, q0 * F:w * F].rearrange(
                                "p (n f) -> p n f", f=F),
                            ACTF.Copy)
                    for (c0, w, psv) in col_chunks:
                        nc.scalar.activation(
                            tl_col[:, 1 + c0:1 + c0 + w, :],
                            psv[:, 0:w * F].rearrange("p (n f) -> p n f", f=F),
                            ACTF.Copy)

                    # prep t data slots
                    if k == 1:
                        nc.gpsimd.memset(tr_cur[:, 1:S, :], 0.0)
                        nc.gpsimd.memset(tc_cur[:, 1:S, :], 0.0)
                    else:
                        trp, tcp = tri(row_prev, k - 1), tri(col_prev, k - 1)
                        # t[j] = max(u_prev[j-1], u_prev[j] * (j-1)/j)
                        # prev data: cell j' at slot j'-(k-1); j=k+1..16:
                        #   u_prev[j]   -> prev slots 2..S_prev-1
                        #   u_prev[j-1] -> prev slots 1..S_prev-2
                        nc.vector.tensor_tensor(
                            tr_cur[:, 1:S, :], trp[:, 2:S + 1, :],
                            rbig[:, k + 1:17, :], ALU.mult)
                        nc.vector.tensor_tensor(
                            tr_cur[:, 1:S, :], tr_cur[:, 1:S, :],
                            trp[:, 1:S, :], ALU.max)
                        nc.vector.tensor_tensor(
                            tc_cur[:, 1:S, :], tcp[:, 2:S + 1, :],
                            rbig[:, k + 1:17, :], ALU.mult)
                        nc.vector.tensor_tensor(
                            tc_cur[:, 1:S, :], tc_cur[:, 1:S, :],
                            tcp[:, 1:S, :], ALU.max)

                    # scans (in place on the t tiles)
                    nc.vector.tensor_tensor_scan(
                        row_cur[:, 0:S * F], row_cur[:, 0:S * F],
                        lrow[:, 0:S * F], 0.0, ALU.max, ALU.add)
                    nc.vector.tensor_tensor_scan(
                        col_cur[:, 0:S * F], col_cur[:, 0:S * F],
                        lcol[:, 0:S * F], 0.0, ALU.max, ALU.add)

                    row_prev, col_prev = row_cur, col_cur

            out_sb = mp.tile([P, F], DT)
            nc.vector.tensor_scalar_mul(out_sb[:, :], ud[T % 2][:, :], 1.0 / T)
            nc.sync.dma_start(out_d.ap(), out_sb[:, :])

    nc.compile()
    return nc


def _consts():
    ident = np.eye(128, dtype=np.float32)
    rb = np.zeros((17, F), dtype=np.float32)
    for m in range(1, 17):
        rb[m, :] = (m - 1) / m
    rbig = np.broadcast_to(rb.reshape(1, 17 * F), (P, 17 * F)).copy()
    return ident, rbig


def kernel(a: np.ndarray, b: np.ndarray) -> np.ndarray:
    a = np.ascontiguousarray(a, dtype=np.float32)
    b = np.ascontiguousarray(b, dtype=np.float32)
    assert a.shape == (NA, T, D) and b.shape == (NB, T, D)

    nc = build_program()
    ident, rbig = _consts()

    in_maps = []
    for core in range(8):
        ca, cb = core // 2, core % 2
        in_maps.append({
            "a_c": a[ca * ACH:(ca + 1) * ACH],
            "b_c": b[cb * BCH:(cb + 1) * BCH],
            "ident": ident,
            "rbig": rbig,
        })

    res = bass_utils.run_bass_kernel_spmd(nc, in_maps, core_ids=list(range(8)))
    global _last_results
    _last_results = res

    out = np.zeros((NA, NB), dtype=np.float32)
    for core in range(8):
        ca, cb = core // 2, core % 2
        out[ca * ACH:(ca + 1) * ACH, cb * BCH:(cb + 1) * BCH] = \
            res.results[core]["out"]
    return out


# revision 16
# speedup vs baseline: 1.0509x; 1.0001x over previous
"""Trainium2 Bass kernel for nn_DynamicMaxSimilarity — scan-based dual-sweep DP.

Full inputs a,b: [512, 16, 256] f32.
  an = l2norm(tanh(a)) rows; bn likewise
  sim[a,b,i,j] = dot(an[a,i], bn[b,j]);  out[a,b] = DTW-like max-avg DP.

Sharding: 8 cores as 4 a-chunks (128) x 2 b-chunks (256). Per-core block
[128 a, 256 b]; pairs live as [128 partitions (a), 256 free (b)].

DP in the scaled domain u[i,j] = si[i,j]*max(i,j):
  step k: diag (k,k); row sweep cells (k, j>k); col sweep cells (i>k, k).
  Row sweep, per cell j: u = max(t[j], u_left) + lc, with
    t[j] = max(u_prev[j-1], u_prev[j]*(j-1)/j)   (prep, bulk)
  done by ONE tensor_tensor_scan (op0=max, op1=add) over free axis
  (pair-major, slots [inject, data...]): the inject slot (d0=diag+BIG,
  d1=-BIG) resets and seeds the per-pair chain. Col sweep symmetric.
Validated exactly vs the reference recurrence in fp64.
"""

import numpy as np

import concourse.bass as bass
from concourse import bacc
import concourse.mybir as mybir
from concourse.tile import TileContext
from concourse import bass_utils

NA, NB, T, D = 512, 512, 16, 256
ACH, BCH = 128, 256
P = 128
F = BCH              # pairs per partition
KH = D // 128
DT = mybir.dt.float32
F32R = mybir.dt.float32r
ALU = mybir.AluOpType
ACTF = mybir.ActivationFunctionType
BIG = 64.0

_last_results = None


def _normalize_block(nc, pool, wp, x_sb, nt, name):
    nc.scalar.activation(x_sb[:, :, :], x_sb[:, :, :], ACTF.Tanh)
    ssq = pool.tile([P, nt], DT, name=f"{name}_ssq")
    sq = wp.tile([P, nt, D], DT, name=f"{name}_sq", tag="sq_scr")
    nc.vector.tensor_tensor(sq[:, :, :], x_sb[:, :, :], x_sb[:, :, :], ALU.mult)
    nc.vector.tensor_reduce(ssq[:, :], sq[:, :, :], mybir.AxisListType.X,
                            ALU.add)
    nrm = pool.tile([P, nt], DT, name=f"{name}_nrm")
    nc.scalar.activation(nrm[:, :], ssq[:, :], ACTF.Sqrt)
    rinv = pool.tile([P, nt], DT, name=f"{name}_rinv")
    nc.vector.reciprocal(rinv[:, :], nrm[:, :])
    for i in range(nt):
        nc.vector.tensor_scalar_mul(x_sb[:, i, :], x_sb[:, i, :],
                                    rinv[:, i:i + 1])
    return x_sb


def build_program():
    nc = bacc.Bacc("TRN2", target_bir_lowering=False, debug=False)

    a_d = nc.dram_tensor("a_c", [ACH, T, D], DT, kind="ExternalInput")
    b_d = nc.dram_tensor("b_c", [BCH, T, D], DT, kind="ExternalInput")
    ident_d = nc.dram_tensor("ident", [128, 128], DT, kind="ExternalInput")
    rbigT_d = nc.dram_tensor("rbigT", [P, 17 * F], DT, kind="ExternalInput")
    out_d = nc.dram_tensor("out", [ACH, BCH], DT, kind="ExternalOutput")

    with TileContext(nc) as tc:
        with (
            tc.tile_pool(name="mp", bufs=1) as mp,
            tc.tile_pool(name="wp", bufs=2) as wp,
        ):
            ident = mp.tile([128, 128], DT)
            nc.sync.dma_start(ident[:, :], ident_d.ap())
            # rbig[m*F + p] = (m-1)/m  (slot-major, constant across pairs)
            rbigT = mp.tile([P, F, 17], DT)
            nc.sync.dma_start(rbigT[:, :, :],
                              rbigT_d.ap().rearrange("p (f m) -> p f m", m=17))

            # ---- load + normalize (in place) ----
            a_sb = mp.tile([P, T, D], DT, tag="sh_lrow")
            for q in range(4):
                nc.sync.dma_start(a_sb[:, q * 4:(q + 1) * 4, :],
                                  a_d.ap()[:, q * 4:(q + 1) * 4, :])
            an = _normalize_block(nc, mp, wp, a_sb, T, "a")

            b_sb = [mp.tile([P, T, D], DT, name=f"b_sb{h}", tag=f"sh_b{h}")
                    for h in range(2)]
            for h in range(2):
                for q in range(4):
                    nc.sync.dma_start(
                        b_sb[h][:, q * 4:(q + 1) * 4, :],
                        b_d.ap()[h * 128:(h + 1) * 128, q * 4:(q + 1) * 4, :])
            bn = [_normalize_block(nc, mp, wp, b_sb[h], T, f"b{h}")
                  for h in range(2)]

            # ---- transposes into matmul layout ----
            anT = mp.tile([P, T, 2, 128], F32R)
            bnT = [mp.tile([P, T, BCH], F32R, name=f"bnT{kh}") for kh in range(KH)]
            with tc.tile_pool(name="tpp", bufs=4, space="PSUM") as tpp:
                def tp_a_group(i0):
                    # 4 transposes (2 i x 2 kh) per PSUM bank, 1 evict
                    ps = tpp.tile([128, 512], DT, name=f"tpa{i0}", tag="tpa")
                    for q, (i, kh) in enumerate(
                            (i0 + di, kh) for di in range(2) for kh in range(KH)):
                        nc.tensor.transpose(ps[:, q * 128:(q + 1) * 128],
                                            an[:, i, kh * 128:(kh + 1) * 128],
                                            ident[:, :])
                    nc.scalar.activation(
                        anT[:, i0:i0 + 2, :, :],
                        ps[:, :].rearrange("p (i k a) -> p i k a", i=2, k=2),
                        ACTF.Copy)

                # frame 0+1 a-group first (gates step 1's row piece), then
                # all b transposes, then the remaining a-groups (only gate
                # the later col pieces).
                tp_a_group(0)
                for j in range(T):
                    for kh in range(KH):
                        ps = tpp.tile([128, 256], DT, name=f"tpb{j}{kh}",
                                      tag="tpb")
                        for bh in range(2):
                            nc.tensor.transpose(
                                ps[:, bh * 128:(bh + 1) * 128],
                                bn[bh][:, j, kh * 128:(kh + 1) * 128],
                                ident[:, :])
                        if j % 2 == 0:
                            nc.scalar.activation(bnT[kh][:, j, :], ps[:, :],
                                                 ACTF.Copy)
                        else:
                            nc.vector.tensor_copy(bnT[kh][:, j, :], ps[:, :])
                for i0 in range(2, T, 2):
                    tp_a_group(i0)

            # ---- DP tiles ----
            # Row and col sweep state are PACKED in one buffer per step:
            # [row region | col region], each S*F (S=17-k, per-pair slots
            # [inject, data...]). One scan per step covers both regions
    	    # back-to-back (the col region's inject slots reset the state
            # crossing the region boundary). a_sb/b_sb memory reused by tags.
            RC0 = mp.tile([P, 2 * 17 * F], DT, tag="sh_lrow")
            RC1 = mp.tile([P, 2 * 17 * F], DT, tag="sh_b0")
            LRC = mp.tile([P, 2 * 17 * F], DT, tag="sh_b1")
            ud = [mp.tile([P, F], DT, name=f"ud{x}") for x in range(2)]

            def reg(tile, k, w):
                """Region w (0=row, 1=col) as [P, slots, pairs] (s, f)."""
                S = 17 - k
                return tile[:, w * S * F:(w + 1) * S * F].rearrange(
                    "p (f s) -> p s f", s=S)

            def reg_fs(tile, k, w):
                """Region w as [P, pairs, slots] (slots contiguous)."""
                S = 17 - k
                return tile[:, w * S * F:(w + 1) * S * F].rearrange(
                    "p (f s) -> p f s", s=S)

            def both(tile, k):
                """[P, region, pairs, slots] 4D view over both regions."""
                S = 17 - k
                return tile[:, 0:2 * S * F].rearrange(
                    "p (w f s) -> p w f s", w=2, s=S)

            rc_prev = None

            # ---- fused matmul + dual-sweep DP ----
            with tc.tile_pool(name="pp", bufs=2, space="PSUM") as pp:
                def mm_planes(psv, i0, j0, np_):
                    """matmul lc planes (frame i0; b-frames j0..j0+np_) into
                    psv [128, np_*F]."""
                    for q0 in range(0, np_, 2):
                        w = min(2, np_ - q0)
                        for kh in range(KH):
                            nc.tensor.matmul(
                                psv[:, q0 * F:(q0 + w) * F],
                                anT[:, i0, kh, :],
                                bnT[kh][:, j0 + q0:j0 + q0 + w, :],
                                start=(kh == 0), stop=(kh == KH - 1),
                            )

                for k in range(1, T + 1):
                    S = 17 - k
                    udp, udc = ud[(k - 1) % 2], ud[k % 2]

                    # --- produce lc L-border for step k ---
                    # diag+row piece: frame k-1 x b-frames k-1..15
                    nrow = 17 - k   # includes diag plane
                    row_chunks = []
                    for c0 in range(0, nrow, 4):
                        w = min(4, nrow - c0)
                        psv = pp.tile([128, 4 * F], DT,
                                      name=f"psr{k}_{c0}", tag="locr")
                        mm_planes(psv, k - 1, (k - 1) + c0, w)
                        row_chunks.append((c0, w, psv))
                    # col piece: frames k..15 x b-frame k-1
                    ncol = 16 - k
                    col_chunks = []
                    for c0 in range(0, ncol, 4):
                        w = min(4, ncol - c0)
                        psv = pp.tile([128, 4 * F], DT,
                                      name=f"psc{k}_{c0}", tag="locc")
                        for q in range(w):
                            for kh in range(KH):
                                nc.tensor.matmul(
                                    psv[:, q * F:(q + 1) * F],
                                    anT[:, k + c0 + q, kh, :],
                                    bnT[kh][:, k - 1:k, :],
                                    start=(kh == 0), stop=(kh == KH - 1),
                                )
                        col_chunks.append((c0, w, psv))

                    # --- diag cell (k,k) ---
                    lc_diag = row_chunks[0][2][:, 0:F]
                    if k == 1:
                        nc.vector.tensor_scalar_add(udc[:, :], lc_diag, 0.0)
                    else:
                        m1 = wp.tile([P, F], DT, name=f"m1_{k}", tag="m1")
                        nc.vector.tensor_tensor(
                            m1[:, :], reg(rc_prev, k - 1, 0)[:, 1, :],
                            reg(rc_prev, k - 1, 1)[:, 1, :], ALU.max)
                        nc.vector.scalar_tensor_tensor(
                            m1[:, :], m1[:, :], float((k - 1) / k), udp[:, :],
                            ALU.mult, ALU.max)
                        nc.vector.tensor_tensor(udc[:, :], m1[:, :], lc_diag,
                                                ALU.add)

                    if k == T:
                        break

                    # --- sweeps ---
                    rc_cur = RC0 if (k % 2) else RC1
                    tr_cur = reg(rc_cur, k, 0)
                    tc_cur = reg(rc_cur, k, 1)
                    tl_row = reg(LRC, k, 0)
                    tl_col = reg(LRC, k, 1)

                    # injects: d0 = diag + BIG (ACT); d1 = -BIG, one merged
                    # memset over both regions' inject slots (gpsimd)
                    nc.scalar.activation(tr_cur[:, 0, :], udc[:, :], ACTF.Copy,
                                         bias=BIG)
                    nc.scalar.activation(tc_cur[:, 0, :], udc[:, :], ACTF.Copy,
                                         bias=BIG)
                    nc.gpsimd.memset(both(LRC, k)[:, :, :, 0], -BIG)

                    # evictions into d1 data slots
                    for (c0, w, psv) in row_chunks:
                        if c0 == 0 and w == 1:
                            continue   # only-diag chunk
                        # chunk covers b-frames (k-1)+c0 .. +w-1; frame p is
                        # cell j=p+1 at d1 slot j-k = p-k+1; skip diag (c0=0).
                        q0 = 1 if c0 == 0 else 0     # local first plane
                        s0 = c0 + q0                 # dest slot of that plane
                        nc.scalar.activation(
                            tl_row[:, s0:c0 + w, :],
                            psv[:, q0 * F:w * F].rearrange(
                                "p (n f) -> p n f", f=F),
                            ACTF.Copy)
                    for (c0, w, psv) in col_chunks:
                        nc.scalar.activation(
                            tl_col[:, 1 + c0:1 + c0 + w, :],
                            psv[:, 0:w * F].rearrange("p (n f) -> p n f", f=F),
                            ACTF.Copy)

                    # prep t data slots
                    if k == 1:
                        # data slots only — the inject slots (slot 0) are
                        # written by the ACT injects; overlapping them here
                        # creates a WAW the scheduler may misorder.
                        nc.gpsimd.memset(both(rc_cur, k)[:, :, :, 1:S], 0.0)
                    else:
                        # t[j] = max(u_prev[j-1], u_prev[j] * (j-1)/j)
                        # prev data: cell j' at slot j'-(k-1); j=k+1..16:
                        #   u_prev[j]   -> prev slots 2..S_prev-1
                        #   u_prev[j-1] -> prev slots 1..S_prev-2
                        # row trio then col trio: step k+1's row chain only
                        # depends on the row scan, so keep each region's
                        # mult->max->scan contiguous.
                        for w in range(2):
                            nc.vector.tensor_tensor(
                                reg_fs(rc_cur, k, w)[:, :, 1:S],
                                reg_fs(rc_prev, k - 1, w)[:, :, 2:S + 1],
                                rbigT[:, :, k + 1:17], ALU.mult)
                            nc.vector.tensor_tensor(
                                reg(rc_cur, k, w)[:, 1:S, :],
                                reg(rc_cur, k, w)[:, 1:S, :],
                                reg(rc_prev, k - 1, w)[:, 1:S, :], ALU.max)
                            nc.vector.tensor_tensor_scan(
                                rc_cur[:, w * S * F:(w + 1) * S * F],
                                rc_cur[:, w * S * F:(w + 1) * S * F],
                                LRC[:, w * S * F:(w + 1) * S * F],
                                0.0, ALU.max, ALU.add)
                    if k == 1:
                        for w in range(2):
                            nc.vector.tensor_tensor_scan(
                                rc_cur[:, w * S * F:(w + 1) * S * F],
                                rc_cur[:, w * S * F:(w + 1) * S * F],
                                LRC[:, w * S * F:(w + 1) * S * F],
                                0.0, ALU.max, ALU.add)

                    rc_prev = rc_cur

            out_sb = mp.tile([P, F], DT)
            nc.vector.tensor_scalar_mul(out_sb[:, :], ud[T % 2][:, :], 1.0 / T)
            nc.sync.dma_start(out_d.ap(), out_sb[:, :])

    nc.compile()
    return nc


def _consts():
    ident = np.eye(128, dtype=np.float32)
    rb = np.zeros((17, F), dtype=np.float32)
    for m in range(1, 17):
        rb[m, :] = (m - 1) / m
    rbT = np.ascontiguousarray(np.broadcast_to(rb[:, 0], (F, 17)))
    rbigT = np.broadcast_to(rbT.reshape(1, F * 17), (P, F * 17)).copy()
    return ident, rbigT


def kernel(a: np.ndarray, b: np.ndarray) -> np.ndarray:
    a = np.ascontiguousarray(a, dtype=np.float32)
    b = np.ascontiguousarray(b, dtype=np.float32)
    assert a.shape == (NA, T, D) and b.shape == (NB, T, D)

    nc = build_program()
    ident, rbigT = _consts()

    in_maps = []
    for core in range(8):
        ca, cb = core // 2, core % 2
        in_maps.append({
            "a_c": a[ca * ACH:(ca + 1) * ACH],
            "b_c": b[cb * BCH:(cb + 1) * BCH],
            "ident": ident,
            "rbigT": rbigT,
        })

    res = bass_utils.run_bass_kernel_spmd(nc, in_maps, core_ids=list(range(8)))
    global _last_results
    _last_results = res

    out = np.zeros((NA, NB), dtype=np.float32)
    for core in range(8):
        ca, cb = core // 2, core % 2
        out[ca * ACH:(ca + 1) * ACH, cb * BCH:(cb + 1) * BCH] = \
            res.results[core]["out"]
    return out


# revision 18
# speedup vs baseline: 1.0643x; 1.0128x over previous
"""Trainium2 Bass kernel for nn_DynamicMaxSimilarity — scan-based dual-sweep DP.

Full inputs a,b: [512, 16, 256] f32.
  an = l2norm(tanh(a)) rows; bn likewise
  sim[a,b,i,j] = dot(an[a,i], bn[b,j]);  out[a,b] = DTW-like max-avg DP.

Sharding: 8 cores as 4 a-chunks (128) x 2 b-chunks (256). Per-core block
[128 a, 256 b]; pairs live as [128 partitions (a), 256 free (b)].

DP in the scaled domain u[i,j] = si[i,j]*max(i,j):
  step k: diag (k,k); row sweep cells (k, j>k); col sweep cells (i>k, k).
  Row sweep, per cell j: u = max(t[j], u_left) + lc, with
    t[j] = max(u_prev[j-1], u_prev[j]*(j-1)/j)   (prep, bulk)
  done by ONE tensor_tensor_scan (op0=max, op1=add) over free axis
  (pair-major, slots [inject, data...]): the inject slot (d0=diag+BIG,
  d1=-BIG) resets and seeds the per-pair chain. Col sweep symmetric.
Validated exactly vs the reference recurrence in fp64.
"""

import numpy as np

import concourse.bass as bass
from concourse import bacc
import concourse.mybir as mybir
from concourse.tile import TileContext
from concourse import bass_utils

NA, NB, T, D = 512, 512, 16, 256
ACH, BCH = 128, 256
P = 128
F = BCH              # pairs per partition
KH = D // 128
DT = mybir.dt.float32
F32R = mybir.dt.float32r
ALU = mybir.AluOpType
ACTF = mybir.ActivationFunctionType
BIG = 64.0

_last_results = None


def _normalize_block(nc, pool, wp, x_sb, nt, name):
    # per-half pipeline: tanh/sumsq of half 1 overlap half 0's tail
    ssq = pool.tile([P, nt], DT, name=f"{name}_ssq")
    h = nt // 2
    for q in range(2):
        sl = slice(q * h, (q + 1) * h)
        nc.scalar.activation(x_sb[:, sl, :], x_sb[:, sl, :], ACTF.Tanh)
        sq = wp.tile([P, h, D], DT, name=f"{name}_sq{q}", tag="sq_scr")
        nc.vector.tensor_tensor(sq[:, :, :], x_sb[:, sl, :], x_sb[:, sl, :],
                                ALU.mult)
        nc.vector.tensor_reduce(ssq[:, sl], sq[:, :, :], mybir.AxisListType.X,
                                ALU.add)
    nrm = pool.tile([P, nt], DT, name=f"{name}_nrm")
    nc.scalar.activation(nrm[:, :], ssq[:, :], ACTF.Sqrt)
    rinv = pool.tile([P, nt], DT, name=f"{name}_rinv")
    nc.vector.reciprocal(rinv[:, :], nrm[:, :])
    for i in range(nt):
        nc.vector.tensor_scalar_mul(x_sb[:, i, :], x_sb[:, i, :],
                                    rinv[:, i:i + 1])
    return x_sb


def build_program():
    nc = bacc.Bacc("TRN2", target_bir_lowering=False, debug=False)

    a_d = nc.dram_tensor("a_c", [ACH, T, D], DT, kind="ExternalInput")
    b_d = nc.dram_tensor("b_c", [BCH, T, D], DT, kind="ExternalInput")
    ident_d = nc.dram_tensor("ident", [128, 128], DT, kind="ExternalInput")
    rbigT_d = nc.dram_tensor("rbigT", [P, 17 * F], DT, kind="ExternalInput")
    out_d = nc.dram_tensor("out", [ACH, BCH], DT, kind="ExternalOutput")

    with TileContext(nc) as tc:
        with (
            tc.tile_pool(name="mp", bufs=1) as mp,
            tc.tile_pool(name="wp", bufs=2) as wp,
        ):
            ident = mp.tile([128, 128], DT)
            nc.sync.dma_start(ident[:, :], ident_d.ap())
            # rbig[m*F + p] = (m-1)/m  (slot-major, constant across pairs)
            rbigT = mp.tile([P, F, 17], DT)
            nc.sync.dma_start(rbigT[:, :, :],
                              rbigT_d.ap().rearrange("p (f m) -> p f m", m=17))

            # ---- load + normalize (in place) ----
            a_sb = mp.tile([P, T, D], DT, tag="sh_lrow")
            for q in range(4):
                nc.sync.dma_start(a_sb[:, q * 4:(q + 1) * 4, :],
                                  a_d.ap()[:, q * 4:(q + 1) * 4, :])
            an = _normalize_block(nc, mp, wp, a_sb, T, "a")

            b_sb = [mp.tile([P, T, D], DT, name=f"b_sb{h}", tag=f"sh_b{h}")
                    for h in range(2)]
            for h in range(2):
                for q in range(4):
                    nc.sync.dma_start(
                        b_sb[h][:, q * 4:(q + 1) * 4, :],
                        b_d.ap()[h * 128:(h + 1) * 128, q * 4:(q + 1) * 4, :])
            bn = [_normalize_block(nc, mp, wp, b_sb[h], T, f"b{h}")
                  for h in range(2)]

            # ---- transposes into matmul layout ----
            anT = mp.tile([P, T, 2, 128], F32R)
            bnT = [mp.tile([P, T, BCH], F32R, name=f"bnT{kh}") for kh in range(KH)]
            with tc.tile_pool(name="tpp", bufs=4, space="PSUM") as tpp:
                def tp_a_group(i0):
                    # 4 transposes (2 i x 2 kh) per PSUM bank, 1 evict
                    ps = tpp.tile([128, 512], DT, name=f"tpa{i0}", tag="tpa")
                    for q, (i, kh) in enumerate(
                            (i0 + di, kh) for di in range(2) for kh in range(KH)):
                        nc.tensor.transpose(ps[:, q * 128:(q + 1) * 128],
                                            an[:, i, kh * 128:(kh + 1) * 128],
                                            ident[:, :])
                    nc.scalar.activation(
                        anT[:, i0:i0 + 2, :, :],
                        ps[:, :].rearrange("p (i k a) -> p i k a", i=2, k=2),
                        ACTF.Copy)

                # frame 0+1 a-group first (gates step 1's row piece), then
                # all b transposes, then the remaining a-groups (only gate
                # the later col pieces).
                tp_a_group(0)
                for j in range(T):
                    for kh in range(KH):
                        ps = tpp.tile([128, 256], DT, name=f"tpb{j}{kh}",
                                      tag="tpb")
                        for bh in range(2):
                            nc.tensor.transpose(
                                ps[:, bh * 128:(bh + 1) * 128],
                                bn[bh][:, j, kh * 128:(kh + 1) * 128],
                                ident[:, :])
                        if j % 2 == 0:
                            nc.scalar.activation(bnT[kh][:, j, :], ps[:, :],
                                                 ACTF.Copy)
                        else:
                            nc.vector.tensor_copy(bnT[kh][:, j, :], ps[:, :])
                for i0 in range(2, T, 2):
                    tp_a_group(i0)

            # ---- DP tiles ----
            # Row and col sweep state are PACKED in one buffer per step:
            # [row region | col region], each S*F (S=17-k, per-pair slots
            # [inject, data...]), scanned per region (a merged scan acts as
            # a barrier and costs ~5us). a_sb/b_sb memory reused by tags.
            RC0 = mp.tile([P, 2 * 17 * F], DT, tag="sh_lrow")
            RC1 = mp.tile([P, 2 * 17 * F], DT, tag="sh_b0")
            LRC = mp.tile([P, 2 * 17 * F], DT, tag="sh_b1")
            ud = [mp.tile([P, F], DT, name=f"ud{x}") for x in range(2)]

            def reg(tile, k, w):
                """Region w (0=row, 1=col) as [P, slots, pairs] (s, f)."""
                S = 17 - k
                return tile[:, w * S * F:(w + 1) * S * F].rearrange(
                    "p (f s) -> p s f", s=S)

            def reg_fs(tile, k, w):
                """Region w as [P, pairs, slots] (slots contiguous)."""
                S = 17 - k
                return tile[:, w * S * F:(w + 1) * S * F].rearrange(
                    "p (f s) -> p f s", s=S)

            def both(tile, k):
                """[P, region, pairs, slots] 4D view over both regions."""
                S = 17 - k
                return tile[:, 0:2 * S * F].rearrange(
                    "p (w f s) -> p w f s", w=2, s=S)

            rc_prev = None

            # ---- fused matmul + dual-sweep DP ----
            with tc.tile_pool(name="pp", bufs=2, space="PSUM") as pp:
                def mm_planes(psv, i0, j0, np_):
                    """matmul lc planes (frame i0; b-frames j0..j0+np_) into
                    psv [128, np_*F]."""
                    for q0 in range(0, np_, 2):
                        w = min(2, np_ - q0)
                        for kh in range(KH):
                            nc.tensor.matmul(
                                psv[:, q0 * F:(q0 + w) * F],
                                anT[:, i0, kh, :],
                                bnT[kh][:, j0 + q0:j0 + q0 + w, :],
                                start=(kh == 0), stop=(kh == KH - 1),
                            )

                for k in range(1, T + 1):
                    S = 17 - k
                    udp, udc = ud[(k - 1) % 2], ud[k % 2]

                    # --- produce lc L-border for step k ---
                    # diag+row piece: frame k-1 x b-frames k-1..15
                    nrow = 17 - k   # includes diag plane
                    row_chunks = []
                    for c0 in range(0, nrow, 4):
                        w = min(4, nrow - c0)
                        psv = pp.tile([128, 4 * F], DT,
                                      name=f"psr{k}_{c0}", tag="locr")
                        mm_planes(psv, k - 1, (k - 1) + c0, w)
                        row_chunks.append((c0, w, psv))
                    # col piece: frames k..15 x b-frame k-1
                    ncol = 16 - k
                    col_chunks = []
                    for c0 in range(0, ncol, 4):
                        w = min(4, ncol - c0)
                        psv = pp.tile([128, 4 * F], DT,
                                      name=f"psc{k}_{c0}", tag="locc")
                        for q in range(w):
                            for kh in range(KH):
                                nc.tensor.matmul(
                                    psv[:, q * F:(q + 1) * F],
                                    anT[:, k + c0 + q, kh, :],
                                    bnT[kh][:, k - 1:k, :],
                                    start=(kh == 0), stop=(kh == KH - 1),
                                )
                        col_chunks.append((c0, w, psv))

                    # --- diag cell (k,k) ---
                    lc_diag = row_chunks[0][2][:, 0:F]
                    if k == 1:
                        nc.vector.tensor_scalar_add(udc[:, :], lc_diag, 0.0)
                    else:
                        m1 = wp.tile([P, F], DT, name=f"m1_{k}", tag="m1")
                        nc.vector.tensor_tensor(
                            m1[:, :], reg(rc_prev, k - 1, 0)[:, 1, :],
                            reg(rc_prev, k - 1, 1)[:, 1, :], ALU.max)
                        nc.vector.scalar_tensor_tensor(
                            m1[:, :], m1[:, :], float((k - 1) / k), udp[:, :],
                            ALU.mult, ALU.max)
                        nc.vector.tensor_tensor(udc[:, :], m1[:, :], lc_diag,
                                                ALU.add)

                    if k == T:
                        break

                    # --- sweeps ---
                    rc_cur = RC0 if (k % 2) else RC1
                    tr_cur = reg(rc_cur, k, 0)
                    tc_cur = reg(rc_cur, k, 1)
                    tl_row = reg(LRC, k, 0)
                    tl_col = reg(LRC, k, 1)

                    # injects: d0 = diag + BIG (ACT); d1 = -BIG, one merged
                    # memset over both regions' inject slots (gpsimd)
                    nc.scalar.activation(tr_cur[:, 0, :], udc[:, :], ACTF.Copy,
                                         bias=BIG)
                    nc.scalar.activation(tc_cur[:, 0, :], udc[:, :], ACTF.Copy,
                                         bias=BIG)
                    nc.gpsimd.memset(both(LRC, k)[:, :, :, 0], -BIG)

                    # evictions into d1 data slots
                    for (c0, w, psv) in row_chunks:
                        if c0 == 0 and w == 1:
                            continue   # only-diag chunk
                        # chunk covers b-frames (k-1)+c0 .. +w-1; frame p is
                        # cell j=p+1 at d1 slot j-k = p-k+1; skip diag (c0=0).
                        q0 = 1 if c0 == 0 else 0     # local first plane
                        s0 = c0 + q0                 # dest slot of that plane
                        nc.scalar.activation(
                            tl_row[:, s0:c0 + w, :],
                            psv[:, q0 * F:w * F].rearrange(
                                "p (n f) -> p n f", f=F),
                            ACTF.Copy)
                    for (c0, w, psv) in col_chunks:
                        nc.scalar.activation(
                            tl_col[:, 1 + c0:1 + c0 + w, :],
                            psv[:, 0:w * F].rearrange("p (n f) -> p n f", f=F),
                            ACTF.Copy)

                    # prep t data slots
                    if k == 1:
                        # data slots only — the inject slots (slot 0) are
                        # written by the ACT injects; overlapping them here
                        # creates a WAW the scheduler may misorder.
                        nc.gpsimd.memset(both(rc_cur, k)[:, :, :, 1:S], 0.0)
                    else:
                        # t[j] = max(u_prev[j-1], u_prev[j] * (j-1)/j)
                        # prev data: cell j' at slot j'-(k-1); j=k+1..16:
                        #   u_prev[j]   -> prev slots 2..S_prev-1
                        #   u_prev[j-1] -> prev slots 1..S_prev-2
                        # row trio then col trio: step k+1's row chain only
                        # depends on the row scan, so keep each region's
                        # mult->max->scan contiguous.
                        for w in range(2):
                            nc.vector.tensor_tensor(
                                reg_fs(rc_cur, k, w)[:, :, 1:S],
                                reg_fs(rc_prev, k - 1, w)[:, :, 2:S + 1],
                                rbigT[:, :, k + 1:17], ALU.mult)
                            nc.vector.tensor_tensor(
                                reg(rc_cur, k, w)[:, 1:S, :],
                                reg(rc_cur, k, w)[:, 1:S, :],
                                reg(rc_prev, k - 1, w)[:, 1:S, :], ALU.max)
                            nc.vector.tensor_tensor_scan(
                                rc_cur[:, w * S * F:(w + 1) * S * F],
                                rc_cur[:, w * S * F:(w + 1) * S * F],
                                LRC[:, w * S * F:(w + 1) * S * F],
                                0.0, ALU.max, ALU.add)
                    if k == 1:
                        for w in range(2):
                            nc.vector.tensor_tensor_scan(
                                rc_cur[:, w * S * F:(w + 1) * S * F],
                                rc_cur[:, w * S * F:(w + 1) * S * F],
                                LRC[:, w * S * F:(w + 1) * S * F],
                                0.0, ALU.max, ALU.add)

                    rc_prev = rc_cur

            out_sb = mp.tile([P, F], DT)
            nc.vector.tensor_scalar_mul(out_sb[:, :], ud[T % 2][:, :], 1.0 / T)
            nc.sync.dma_start(out_d.ap(), out_sb[:, :])

    nc.compile()
    return nc


def _consts():
    ident = np.eye(128, dtype=np.float32)
    rb = np.zeros((17, F), dtype=np.float32)
    for m in range(1, 17):
        rb[m, :] = (m - 1) / m
    rbT = np.ascontiguousarray(np.broadcast_to(rb[:, 0], (F, 17)))
    rbigT = np.broadcast_to(rbT.reshape(1, F * 17), (P, F * 17)).copy()
    return ident, rbigT


def kernel(a: np.ndarray, b: np.ndarray) -> np.ndarray:
    a = np.ascontiguousarray(a, dtype=np.float32)
    b = np.ascontiguousarray(b, dtype=np.float32)
    assert a.shape == (NA, T, D) and b.shape == (NB, T, D)

    nc = build_program()
    ident, rbigT = _consts()

    in_maps = []
    for core in range(8):
        ca, cb = core // 2, core % 2
        in_maps.append({
            "a_c": a[ca * ACH:(ca + 1) * ACH],
            "b_c": b[cb * BCH:(cb + 1) * BCH],
            "ident": ident,
            "rbigT": rbigT,
        })

    res = bass_utils.run_bass_kernel_spmd(nc, in_maps, core_ids=list(range(8)))
    global _last_results
    _last_results = res

    out = np.zeros((NA, NB), dtype=np.float32)
    for core in range(8):
        ca, cb = core // 2, core % 2
        out[ca * ACH:(ca + 1) * ACH, cb * BCH:(cb + 1) * BCH] = \
            res.results[core]["out"]
    return out


# revision 19
# speedup vs baseline: 1.0804x; 1.0151x over previous
"""Trainium2 Bass kernel for nn_DynamicMaxSimilarity — scan-based dual-sweep DP.

Full inputs a,b: [512, 16, 256] f32.
  an = l2norm(tanh(a)) rows; bn likewise
  sim[a,b,i,j] = dot(an[a,i], bn[b,j]);  out[a,b] = DTW-like max-avg DP.

Sharding: 8 cores as 4 a-chunks (128) x 2 b-chunks (256). Per-core block
[128 a, 256 b]; pairs live as [128 partitions (a), 256 free (b)].

DP in the scaled domain u[i,j] = si[i,j]*max(i,j):
  step k: diag (k,k); row sweep cells (k, j>k); col sweep cells (i>k, k).
  Row sweep, per cell j: u = max(t[j], u_left) + lc, with
    t[j] = max(u_prev[j-1], u_prev[j]*(j-1)/j)   (prep, bulk)
  done by ONE tensor_tensor_scan (op0=max, op1=add) over free axis
  (pair-major, slots [inject, data...]): the inject slot (d0=diag+BIG,
  d1=-BIG) resets and seeds the per-pair chain. Col sweep symmetric.
Validated exactly vs the reference recurrence in fp64.
"""

import numpy as np

import concourse.bass as bass
from concourse import bacc
import concourse.mybir as mybir
from concourse.tile import TileContext
from concourse import bass_utils

NA, NB, T, D = 512, 512, 16, 256
ACH, BCH = 128, 256
P = 128
F = BCH              # pairs per partition
KH = D // 128
DT = mybir.dt.float32
F32R = mybir.dt.float32r
ALU = mybir.AluOpType
ACTF = mybir.ActivationFunctionType
BIG = 64.0

_last_results = None


def _normalize_block(nc, pool, wp, x_sb, nt, name):
    # per-half pipeline: tanh/sumsq of half 1 overlap half 0's tail
    ssq = pool.tile([P, nt], DT, name=f"{name}_ssq")
    h = nt // 2
    for q in range(2):
        sl = slice(q * h, (q + 1) * h)
        nc.scalar.activation(x_sb[:, sl, :], x_sb[:, sl, :], ACTF.Tanh)
        sq = wp.tile([P, h, D], DT, name=f"{name}_sq{q}", tag="sq_scr")
        nc.vector.tensor_tensor(sq[:, :, :], x_sb[:, sl, :], x_sb[:, sl, :],
                                ALU.mult)
        nc.vector.tensor_reduce(ssq[:, sl], sq[:, :, :], mybir.AxisListType.X,
                                ALU.add)
    nrm = pool.tile([P, nt], DT, name=f"{name}_nrm")
    nc.scalar.activation(nrm[:, :], ssq[:, :], ACTF.Sqrt)
    rinv = pool.tile([P, nt], DT, name=f"{name}_rinv")
    nc.vector.reciprocal(rinv[:, :], nrm[:, :])
    for i in range(nt):
        nc.vector.tensor_scalar_mul(x_sb[:, i, :], x_sb[:, i, :],
                                    rinv[:, i:i + 1])
    return x_sb


def build_program():
    nc = bacc.Bacc("TRN2", target_bir_lowering=False, debug=False)

    a_d = nc.dram_tensor("a_c", [ACH, T, D], DT, kind="ExternalInput")
    b_d = nc.dram_tensor("b_c", [BCH, T, D], DT, kind="ExternalInput")
    ident_d = nc.dram_tensor("ident", [128, 128], DT, kind="ExternalInput")
    rbigT_d = nc.dram_tensor("rbigT", [P, 17 * F], DT, kind="ExternalInput")
    out_d = nc.dram_tensor("out", [ACH, BCH], DT, kind="ExternalOutput")

    with TileContext(nc) as tc:
        with (
            tc.tile_pool(name="mp", bufs=1) as mp,
            tc.tile_pool(name="wp", bufs=2) as wp,
        ):
            ident = mp.tile([128, 128], DT)
            nc.sync.dma_start(ident[:, :], ident_d.ap())
            # rbig[m*F + p] = (m-1)/m  (slot-major, constant across pairs)
            rbigT = mp.tile([P, F, 17], DT)
            nc.sync.dma_start(rbigT[:, :, :],
                              rbigT_d.ap().rearrange("p (f m) -> p f m", m=17))

            # ---- load + normalize (in place) ----
            a_sb = mp.tile([P, T, D], DT, tag="sh_lrow")
            for q in range(4):
                nc.sync.dma_start(a_sb[:, q * 4:(q + 1) * 4, :],
                                  a_d.ap()[:, q * 4:(q + 1) * 4, :])
            an = _normalize_block(nc, mp, wp, a_sb, T, "a")

            b_sb = [mp.tile([P, T, D], DT, name=f"b_sb{h}", tag=f"sh_b{h}")
                    for h in range(2)]
            for h in range(2):
                for q in range(4):
                    nc.sync.dma_start(
                        b_sb[h][:, q * 4:(q + 1) * 4, :],
                        b_d.ap()[h * 128:(h + 1) * 128, q * 4:(q + 1) * 4, :])
            bn = [_normalize_block(nc, mp, wp, b_sb[h], T, f"b{h}")
                  for h in range(2)]

            # ---- transposes into matmul layout ----
            anT = mp.tile([P, T, 2, 128], F32R)
            bnT = [mp.tile([P, T, BCH], F32R, name=f"bnT{kh}") for kh in range(KH)]
            with tc.tile_pool(name="tpp", bufs=4, space="PSUM") as tpp:
                def tp_a_group(i0):
                    # 4 transposes (2 i x 2 kh) per PSUM bank, 1 evict
                    ps = tpp.tile([128, 512], DT, name=f"tpa{i0}", tag="tpa")
                    for q, (i, kh) in enumerate(
                            (i0 + di, kh) for di in range(2) for kh in range(KH)):
                        nc.tensor.transpose(ps[:, q * 128:(q + 1) * 128],
                                            an[:, i, kh * 128:(kh + 1) * 128],
                                            ident[:, :])
                    nc.scalar.activation(
                        anT[:, i0:i0 + 2, :, :],
                        ps[:, :].rearrange("p (i k a) -> p i k a", i=2, k=2),
                        ACTF.Copy)

                # frame 0+1 a-group first (gates step 1's row piece), then
                # all b transposes, then the remaining a-groups (only gate
                # the later col pieces).
                tp_a_group(0)
                for j in range(T):
                    for kh in range(KH):
                        ps = tpp.tile([128, 256], DT, name=f"tpb{j}{kh}",
                                      tag="tpb")
                        for bh in range(2):
                            nc.tensor.transpose(
                                ps[:, bh * 128:(bh + 1) * 128],
                                bn[bh][:, j, kh * 128:(kh + 1) * 128],
                                ident[:, :])
                        if j % 2 == 0:
                            nc.scalar.activation(bnT[kh][:, j, :], ps[:, :],
                                                 ACTF.Copy)
                        else:
                            nc.vector.tensor_copy(bnT[kh][:, j, :], ps[:, :])
                for i0 in range(2, T, 2):
                    tp_a_group(i0)

            # ---- DP tiles ----
            # Row and col sweep state are PACKED in one buffer per step:
            # [row region | col region], each S*F (S=17-k, per-pair slots
            # [inject, data...]), scanned per region (a merged scan acts as
            # a barrier and costs ~5us). a_sb/b_sb memory reused by tags.
            # per-pair slots: [D1, D2, data j=k+1..16]; Sl = 18-k.
            # D1 (d0=udp+BIG, d1=-BIG) resets state to the previous diag;
            # D2 (d0=max(u[k-1,k],u[k,k-1])*(k-1)/k, d1=lc[k,k]) computes the
            # step-k diag INSIDE the scan; its output slot is next step's udp.
            RC0 = mp.tile([P, 2 * 18 * F], DT, tag="sh_lrow")
            RC1 = mp.tile([P, 2 * 18 * F], DT, tag="sh_b0")
            LRC = mp.tile([P, 2 * 18 * F], DT, tag="sh_b1")
            ud = [mp.tile([P, F], DT, name=f"ud{x}") for x in range(2)]

            def reg(tile, k, w):
                """Region w (0=row, 1=col) as [P, slots, pairs] (s, f)."""
                S = 18 - k
                return tile[:, w * S * F:(w + 1) * S * F].rearrange(
                    "p (f s) -> p s f", s=S)

            def reg_fs(tile, k, w):
                """Region w as [P, pairs, slots] (slots contiguous)."""
                S = 18 - k
                return tile[:, w * S * F:(w + 1) * S * F].rearrange(
                    "p (f s) -> p f s", s=S)

            def both(tile, k):
                """[P, region, pairs, slots] 4D view over both regions."""
                S = 18 - k
                return tile[:, 0:2 * S * F].rearrange(
                    "p (w f s) -> p w f s", w=2, s=S)

            rc_prev = None

            # ---- fused matmul + dual-sweep DP ----
            with tc.tile_pool(name="pp", bufs=2, space="PSUM") as pp:
                def mm_planes(psv, i0, j0, np_):
                    """matmul lc planes (frame i0; b-frames j0..j0+np_) into
                    psv [128, np_*F]."""
                    for q0 in range(0, np_, 2):
                        w = min(2, np_ - q0)
                        for kh in range(KH):
                            nc.tensor.matmul(
                                psv[:, q0 * F:(q0 + w) * F],
                                anT[:, i0, kh, :],
                                bnT[kh][:, j0 + q0:j0 + q0 + w, :],
                                start=(kh == 0), stop=(kh == KH - 1),
                            )

                for k in range(1, T + 1):
                    S = 18 - k   # slots per pair: [D1, D2, data...]
                    udp, udc = ud[(k - 1) % 2], ud[k % 2]

                    # --- produce lc L-border for step k ---
                    # diag+row piece: frame k-1 x b-frames k-1..15
                    nrow = 17 - k   # includes diag plane
                    row_chunks = []
                    for c0 in range(0, nrow, 4):
                        w = min(4, nrow - c0)
                        psv = pp.tile([128, 4 * F], DT,
                                      name=f"psr{k}_{c0}", tag="locr")
                        mm_planes(psv, k - 1, (k - 1) + c0, w)
                        row_chunks.append((c0, w, psv))
                    # col piece: frames k..15 x b-frame k-1
                    ncol = 16 - k
                    col_chunks = []
                    for c0 in range(0, ncol, 4):
                        w = min(4, ncol - c0)
                        psv = pp.tile([128, 4 * F], DT,
                                      name=f"psc{k}_{c0}", tag="locc")
                        for q in range(w):
                            for kh in range(KH):
                                nc.tensor.matmul(
                                    psv[:, q * F:(q + 1) * F],
                                    anT[:, k + c0 + q, kh, :],
                                    bnT[kh][:, k - 1:k, :],
                                    start=(kh == 0), stop=(kh == KH - 1),
                                )
                        col_chunks.append((c0, w, psv))

                    # --- diag cell (k,k) ---
                    lc_diag = row_chunks[0][2][:, 0:F]
                    m1 = None
                    if k > 1:
                        m1 = wp.tile([P, F], DT, name=f"m1_{k}", tag="m1")
                        nc.vector.tensor_tensor(
                            m1[:, :], reg(rc_prev, k - 1, 0)[:, 2, :],
                            reg(rc_prev, k - 1, 1)[:, 2, :], ALU.max)
                    if k == T:
                        # no scans at k=16: explicit diag
                        udpT = reg(rc_prev, k - 1, 0)[:, 1, :]
                        nc.vector.scalar_tensor_tensor(
                            m1[:, :], m1[:, :], float((k - 1) / k), udpT,
                            ALU.mult, ALU.max)
                        nc.vector.tensor_tensor(udc[:, :], m1[:, :], lc_diag,
                                                ALU.add)
                        break

                    # --- sweeps ---
                    rc_cur = RC0 if (k % 2) else RC1
                    tr_cur = reg(rc_cur, k, 0)
                    tc_cur = reg(rc_cur, k, 1)
                    tl_row = reg(LRC, k, 0)
                    tl_col = reg(LRC, k, 1)

                    # injects. D1: d0 = udp + BIG (prev diag from the prev
                    # row scan's D2 slot), d1 = -BIG. D2: d0 = m1*(k-1)/k
                    # (ACT scaled copy), d1 = lc_diag (rides the evictions).
                    if k == 1:
                        nc.gpsimd.memset(both(rc_cur, k)[:, :, :, 0], BIG)
                    else:
                        udp_sl = reg(rc_prev, k - 1, 0)[:, 1, :]
                        nc.scalar.activation(tr_cur[:, 0, :], udp_sl,
                                             ACTF.Copy, bias=BIG)
                        nc.scalar.activation(tc_cur[:, 0, :], udp_sl,
                                             ACTF.Copy, bias=BIG)
                        nc.scalar.activation(tr_cur[:, 1, :], m1[:, :],
                                             ACTF.Copy,
                                             scale=float((k - 1) / k))
                        nc.scalar.activation(tc_cur[:, 1, :], m1[:, :],
                                             ACTF.Copy,
                                             scale=float((k - 1) / k))
                    nc.gpsimd.memset(both(LRC, k)[:, :, :, 0], -BIG)
                    # col region's D2 d1 = lc_diag (row region's comes with
                    # the row chunk eviction below)
                    nc.scalar.activation(tl_col[:, 1, :], lc_diag, ACTF.Copy)

                    # evictions into d1 slots: b-frame plane p lands at
                    # slot p-k+2 (the diag plane p=k-1 lands at D2=slot 1).
                    for (c0, w, psv) in row_chunks:
                        nc.scalar.activation(
                            tl_row[:, 1 + c0:1 + c0 + w, :],
                            psv[:, 0:w * F].rearrange("p (n f) -> p n f", f=F),
                            ACTF.Copy)
                    for (c0, w, psv) in col_chunks:
                        nc.scalar.activation(
                            tl_col[:, 2 + c0:2 + c0 + w, :],
                            psv[:, 0:w * F].rearrange("p (n f) -> p n f", f=F),
                            ACTF.Copy)

                    # prep t data slots (data at slots 2..S-1)
                    if k == 1:
                        # D2 + data slots = 0 (disjoint from the D1 writes)
                        nc.gpsimd.memset(both(rc_cur, k)[:, :, :, 1:S], 0.0)
                        for w in range(2):
                            nc.vector.tensor_tensor_scan(
                                rc_cur[:, w * S * F:(w + 1) * S * F],
                                rc_cur[:, w * S * F:(w + 1) * S * F],
                                LRC[:, w * S * F:(w + 1) * S * F],
                                0.0, ALU.max, ALU.add)
                    else:
                        # t[j] = max(u_prev[j-1], u_prev[j] * (j-1)/j)
                        # prev cell j' at slot j'-k+3; j=k+1..16:
                        #   u_prev[j]   -> prev slots 4..S_prev-1... wait
                        #   u_prev[j]   -> prev slots j-k+3 = 4..S_prev+... 
                        # (S_prev = 19-k): u_prev[j] slots 4..18-k? No:
                        # j=k+1 -> slot 4?? j-(k-1)+1 with data base 2:
                        # prev cell j' maps to slot j'-(k-1)+1 = j'-k+2.
                        # u_prev[j], j=k+1..16 -> prev slots 3..18-k
                        # u_prev[j-1], j-1=k..15 -> prev slots 2..17-k
                        for w in range(2):
                            nc.vector.tensor_tensor(
                                reg_fs(rc_cur, k, w)[:, :, 2:S],
                                reg_fs(rc_prev, k - 1, w)[:, :, 3:S + 1],
                                rbigT[:, :, k + 1:17], ALU.mult)
                            nc.vector.tensor_tensor(
                                reg(rc_cur, k, w)[:, 2:S, :],
                                reg(rc_cur, k, w)[:, 2:S, :],
                                reg(rc_prev, k - 1, w)[:, 2:S, :], ALU.max)
                            nc.vector.tensor_tensor_scan(
                                rc_cur[:, w * S * F:(w + 1) * S * F],
                                rc_cur[:, w * S * F:(w + 1) * S * F],
                                LRC[:, w * S * F:(w + 1) * S * F],
                                0.0, ALU.max, ALU.add)

                    rc_prev = rc_cur

            out_sb = mp.tile([P, F], DT)
            nc.vector.tensor_scalar_mul(out_sb[:, :], ud[T % 2][:, :], 1.0 / T)
            nc.sync.dma_start(out_d.ap(), out_sb[:, :])

    nc.compile()
    return nc


def _consts():
    ident = np.eye(128, dtype=np.float32)
    rb = np.zeros((17, F), dtype=np.float32)
    for m in range(1, 17):
        rb[m, :] = (m - 1) / m
    rbT = np.ascontiguousarray(np.broadcast_to(rb[:, 0], (F, 17)))
    rbigT = np.broadcast_to(rbT.reshape(1, F * 17), (P, F * 17)).copy()
    return ident, rbigT


def kernel(a: np.ndarray, b: np.ndarray) -> np.ndarray:
    a = np.ascontiguousarray(a, dtype=np.float32)
    b = np.ascontiguousarray(b, dtype=np.float32)
    assert a.shape == (NA, T, D) and b.shape == (NB, T, D)

    nc = build_program()
    ident, rbigT = _consts()

    in_maps = []
    for core in range(8):
        ca, cb = core // 2, core % 2
        in_maps.append({
            "a_c": a[ca * ACH:(ca + 1) * ACH],
            "b_c": b[cb * BCH:(cb + 1) * BCH],
            "ident": ident,
            "rbigT": rbigT,
        })

    res = bass_utils.run_bass_kernel_spmd(nc, in_maps, core_ids=list(range(8)))
    global _last_results
    _last_results = res

    out = np.zeros((NA, NB), dtype=np.float32)
    for core in range(8):
        ca, cb = core // 2, core % 2
        out[ca * ACH:(ca + 1) * ACH, cb * BCH:(cb + 1) * BCH] = \
            res.results[core]["out"]
    return out


# revision 20
# speedup vs baseline: 1.0975x; 1.0158x over previous
"""Trainium2 Bass kernel for nn_DynamicMaxSimilarity — scan-based dual-sweep DP.

Full inputs a,b: [512, 16, 256] f32.
  an = l2norm(tanh(a)) rows; bn likewise
  sim[a,b,i,j] = dot(an[a,i], bn[b,j]);  out[a,b] = DTW-like max-avg DP.

Sharding: 8 cores as 4 a-chunks (128) x 2 b-chunks (256). Per-core block
[128 a, 256 b]; pairs live as [128 partitions (a), 256 free (b)].

DP in the scaled domain u[i,j] = si[i,j]*max(i,j):
  step k: diag (k,k); row sweep cells (k, j>k); col sweep cells (i>k, k).
  Row sweep, per cell j: u = max(t[j], u_left) + lc, with
    t[j] = max(u_prev[j-1], u_prev[j]*(j-1)/j)   (prep, bulk)
  done by ONE tensor_tensor_scan (op0=max, op1=add) over free axis
  (pair-major, slots [inject, data...]): the inject slot (d0=diag+BIG,
  d1=-BIG) resets and seeds the per-pair chain. Col sweep symmetric.
Validated exactly vs the reference recurrence in fp64.
"""

import numpy as np

import concourse.bass as bass
from concourse import bacc
import concourse.mybir as mybir
from concourse.tile import TileContext
from concourse import bass_utils

NA, NB, T, D = 512, 512, 16, 256
ACH, BCH = 128, 256
P = 128
F = BCH              # pairs per partition
KH = D // 128
DT = mybir.dt.float32
F32R = mybir.dt.float32r
ALU = mybir.AluOpType
ACTF = mybir.ActivationFunctionType
BIG = 64.0

_last_results = None


def _normalize_block(nc, pool, wp, x_sb, nt, name):
    # per-quarter pipeline: tanh/sumsq chunks overlap the DMA tail and
    # each other across ACT/DVE
    ssq = pool.tile([P, nt], DT, name=f"{name}_ssq")
    nrm = pool.tile([P, nt], DT, name=f"{name}_nrm")
    rinv = pool.tile([P, nt], DT, name=f"{name}_rinv")
    h = nt // 4
    for q in range(4):
        sl = slice(q * h, (q + 1) * h)
        nc.scalar.activation(x_sb[:, sl, :], x_sb[:, sl, :], ACTF.Tanh)
        sq = wp.tile([P, h, D], DT, name=f"{name}_sq{q}", tag="sq_scr")
        nc.vector.tensor_tensor(sq[:, :, :], x_sb[:, sl, :], x_sb[:, sl, :],
                                ALU.mult)
        nc.vector.tensor_reduce(ssq[:, sl], sq[:, :, :], mybir.AxisListType.X,
                                ALU.add)
        # per-quarter norm finish so early quarters' scales don't wait on
        # the last quarter's sumsq
        nc.scalar.activation(nrm[:, sl], ssq[:, sl], ACTF.Sqrt)
        nc.vector.reciprocal(rinv[:, sl], nrm[:, sl])
        for i in range(q * h, (q + 1) * h):
            nc.vector.tensor_scalar_mul(x_sb[:, i, :], x_sb[:, i, :],
                                        rinv[:, i:i + 1])
    return x_sb


def build_program():
    nc = bacc.Bacc("TRN2", target_bir_lowering=False, debug=False)

    a_d = nc.dram_tensor("a_c", [ACH, T, D], DT, kind="ExternalInput")
    b_d = nc.dram_tensor("b_c", [BCH, T, D], DT, kind="ExternalInput")
    ident_d = nc.dram_tensor("ident", [128, 128], DT, kind="ExternalInput")
    rbigT_d = nc.dram_tensor("rbigT", [P, 17 * F], DT, kind="ExternalInput")
    out_d = nc.dram_tensor("out", [ACH, BCH], DT, kind="ExternalOutput")

    with TileContext(nc) as tc:
        with (
            tc.tile_pool(name="mp", bufs=1) as mp,
            tc.tile_pool(name="wp", bufs=2) as wp,
        ):
            ident = mp.tile([128, 128], DT)
            nc.sync.dma_start(ident[:, :], ident_d.ap())
            # rbig[m*F + p] = (m-1)/m  (slot-major, constant across pairs)
            rbigT = mp.tile([P, F, 17], DT)
            nc.sync.dma_start(rbigT[:, :, :],
                              rbigT_d.ap().rearrange("p (f m) -> p f m", m=17))

            # ---- load + normalize (in place) ----
            a_sb = mp.tile([P, T, D], DT, tag="sh_lrow")
            for q in range(4):
                nc.sync.dma_start(a_sb[:, q * 4:(q + 1) * 4, :],
                                  a_d.ap()[:, q * 4:(q + 1) * 4, :])
            an = _normalize_block(nc, mp, wp, a_sb, T, "a")

            b_sb = [mp.tile([P, T, D], DT, name=f"b_sb{h}", tag=f"sh_b{h}")
                    for h in range(2)]
            for h in range(2):
                for q in range(4):
                    nc.sync.dma_start(
                        b_sb[h][:, q * 4:(q + 1) * 4, :],
                        b_d.ap()[h * 128:(h + 1) * 128, q * 4:(q + 1) * 4, :])
            bn = [_normalize_block(nc, mp, wp, b_sb[h], T, f"b{h}")
                  for h in range(2)]

            # ---- transposes into matmul layout ----
            anT = mp.tile([P, T, 2, 128], F32R)
            bnT = [mp.tile([P, T, BCH], F32R, name=f"bnT{kh}") for kh in range(KH)]
            with tc.tile_pool(name="tpp", bufs=4, space="PSUM") as tpp:
                def tp_a_group(i0):
                    # 4 transposes (2 i x 2 kh) per PSUM bank, 1 evict
                    ps = tpp.tile([128, 512], DT, name=f"tpa{i0}", tag="tpa",
                                  bufs=2)
                    for q, (i, kh) in enumerate(
                            (i0 + di, kh) for di in range(2) for kh in range(KH)):
                        nc.tensor.transpose(ps[:, q * 128:(q + 1) * 128],
                                            an[:, i, kh * 128:(kh + 1) * 128],
                                            ident[:, :])
                    nc.scalar.activation(
                        anT[:, i0:i0 + 2, :, :],
                        ps[:, :].rearrange("p (i k a) -> p i k a", i=2, k=2),
                        ACTF.Copy)

                # frame 0+1 a-group first (gates step 1's row piece), then
                # all b transposes, then the remaining a-groups (only gate
                # the later col pieces).
                tp_a_group(0)
                for j in range(T):
                    for kh in range(KH):
                        ps = tpp.tile([128, 256], DT, name=f"tpb{j}{kh}",
                                      tag="tpb", bufs=6)
                        for bh in range(2):
                            nc.tensor.transpose(
                                ps[:, bh * 128:(bh + 1) * 128],
                                bn[bh][:, j, kh * 128:(kh + 1) * 128],
                                ident[:, :])
                        if j % 2 == 0:
                            nc.scalar.activation(bnT[kh][:, j, :], ps[:, :],
                                                 ACTF.Copy)
                        else:
                            nc.vector.tensor_copy(bnT[kh][:, j, :], ps[:, :])
                for i0 in range(2, T, 2):
                    tp_a_group(i0)

            # ---- DP tiles ----
            # Row and col sweep state are PACKED in one buffer per step:
            # [row region | col region], each S*F (S=17-k, per-pair slots
            # [inject, data...]), scanned per region (a merged scan acts as
            # a barrier and costs ~5us). a_sb/b_sb memory reused by tags.
            # per-pair slots: [D1, D2, data j=k+1..16]; Sl = 18-k.
            # D1 (d0=udp+BIG, d1=-BIG) resets state to the previous diag;
            # D2 (d0=max(u[k-1,k],u[k,k-1])*(k-1)/k, d1=lc[k,k]) computes the
            # step-k diag INSIDE the scan; its output slot is next step's udp.
            RC0 = mp.tile([P, 2 * 18 * F], DT, tag="sh_lrow")
            RC1 = mp.tile([P, 2 * 18 * F], DT, tag="sh_b0")
            LRC = mp.tile([P, 2 * 18 * F], DT, tag="sh_b1")
            ud = [mp.tile([P, F], DT, name=f"ud{x}") for x in range(2)]

            def reg(tile, k, w):
                """Region w (0=row, 1=col) as [P, slots, pairs] (s, f)."""
                S = 18 - k
                return tile[:, w * S * F:(w + 1) * S * F].rearrange(
                    "p (f s) -> p s f", s=S)

            def reg_fs(tile, k, w):
                """Region w as [P, pairs, slots] (slots contiguous)."""
                S = 18 - k
                return tile[:, w * S * F:(w + 1) * S * F].rearrange(
                    "p (f s) -> p f s", s=S)

            def both(tile, k):
                """[P, region, pairs, slots] 4D view over both regions."""
                S = 18 - k
                return tile[:, 0:2 * S * F].rearrange(
                    "p (w f s) -> p w f s", w=2, s=S)

            rc_prev = None

            # ---- fused matmul + dual-sweep DP ----
            with tc.tile_pool(name="pp", bufs=2, space="PSUM") as pp:
                def mm_planes(psv, i0, j0, np_):
                    """matmul lc planes (frame i0; b-frames j0..j0+np_) into
                    psv [128, np_*F]."""
                    for q0 in range(0, np_, 2):
                        w = min(2, np_ - q0)
                        for kh in range(KH):
                            nc.tensor.matmul(
                                psv[:, q0 * F:(q0 + w) * F],
                                anT[:, i0, kh, :],
                                bnT[kh][:, j0 + q0:j0 + q0 + w, :],
                                start=(kh == 0), stop=(kh == KH - 1),
                            )

                for k in range(1, T + 1):
                    S = 18 - k   # slots per pair: [D1, D2, data...]
                    udp, udc = ud[(k - 1) % 2], ud[k % 2]

                    # --- produce lc L-border for step k ---
                    # diag+row piece: frame k-1 x b-frames k-1..15
                    nrow = 17 - k   # includes diag plane
                    row_chunks = []
                    for c0 in range(0, nrow, 4):
                        w = min(4, nrow - c0)
                        psv = pp.tile([128, 4 * F], DT,
                                      name=f"psr{k}_{c0}", tag="locr")
                        mm_planes(psv, k - 1, (k - 1) + c0, w)
                        row_chunks.append((c0, w, psv))
                    # col piece: frames k..15 x b-frame k-1
                    ncol = 16 - k
                    col_chunks = []
                    for c0 in range(0, ncol, 4):
                        w = min(4, ncol - c0)
                        psv = pp.tile([128, 4 * F], DT,
                                      name=f"psc{k}_{c0}", tag="locc")
                        for q in range(w):
                            for kh in range(KH):
                                nc.tensor.matmul(
                                    psv[:, q * F:(q + 1) * F],
                                    anT[:, k + c0 + q, kh, :],
                                    bnT[kh][:, k - 1:k, :],
                                    start=(kh == 0), stop=(kh == KH - 1),
                                )
                        col_chunks.append((c0, w, psv))

                    # --- diag cell (k,k) ---
                    lc_diag = row_chunks[0][2][:, 0:F]
                    m1 = None
                    if k > 1:
                        m1 = wp.tile([P, F], DT, name=f"m1_{k}", tag="m1")
                        nc.vector.tensor_tensor(
                            m1[:, :], reg(rc_prev, k - 1, 0)[:, 2, :],
                            reg(rc_prev, k - 1, 1)[:, 2, :], ALU.max)
                    if k == T:
                        # no scans at k=16: explicit diag
                        udpT = reg(rc_prev, k - 1, 0)[:, 1, :]
                        nc.vector.scalar_tensor_tensor(
                            m1[:, :], m1[:, :], float((k - 1) / k), udpT,
                            ALU.mult, ALU.max)
                        nc.vector.tensor_tensor(udc[:, :], m1[:, :], lc_diag,
                                                ALU.add)
                        break

                    # --- sweeps ---
                    rc_cur = RC0 if (k % 2) else RC1
                    tr_cur = reg(rc_cur, k, 0)
                    tc_cur = reg(rc_cur, k, 1)
                    tl_row = reg(LRC, k, 0)
                    tl_col = reg(LRC, k, 1)

                    # injects. D1: d0 = udp + BIG (prev diag from the prev
                    # row scan's D2 slot), d1 = -BIG. D2: d0 = m1*(k-1)/k
                    # (ACT scaled copy), d1 = lc_diag (rides the evictions).
                    if k == 1:
                        nc.gpsimd.memset(both(rc_cur, k)[:, :, :, 0], BIG)
                    else:
                        udp_sl = reg(rc_prev, k - 1, 0)[:, 1, :]
                        nc.scalar.activation(tr_cur[:, 0, :], udp_sl,
                                             ACTF.Copy, bias=BIG)
                        nc.scalar.activation(tc_cur[:, 0, :], udp_sl,
                                             ACTF.Copy, bias=BIG)
                        nc.scalar.activation(tr_cur[:, 1, :], m1[:, :],
                                             ACTF.Copy,
                                             scale=float((k - 1) / k))
                        nc.scalar.activation(tc_cur[:, 1, :], m1[:, :],
                                             ACTF.Copy,
                                             scale=float((k - 1) / k))
                    nc.gpsimd.memset(both(LRC, k)[:, :, :, 0], -BIG)
                    # col region's D2 d1 = lc_diag (row region's comes with
                    # the row chunk eviction below)
                    nc.scalar.activation(tl_col[:, 1, :], lc_diag, ACTF.Copy)

                    # evictions into d1 slots: b-frame plane p lands at
                    # slot p-k+2 (the diag plane p=k-1 lands at D2=slot 1).
                    for (c0, w, psv) in row_chunks:
                        nc.scalar.activation(
                            tl_row[:, 1 + c0:1 + c0 + w, :],
                            psv[:, 0:w * F].rearrange("p (n f) -> p n f", f=F),
                            ACTF.Copy)
                    for (c0, w, psv) in col_chunks:
                        nc.scalar.activation(
                            tl_col[:, 2 + c0:2 + c0 + w, :],
                            psv[:, 0:w * F].rearrange("p (n f) -> p n f", f=F),
                            ACTF.Copy)

                    # prep t data slots (data at slots 2..S-1)
                    if k == 1:
                        # D2 + data slots = 0 (disjoint from the D1 writes)
                        nc.gpsimd.memset(both(rc_cur, k)[:, :, :, 1:S], 0.0)
                        for w in range(2):
                            nc.vector.tensor_tensor_scan(
                                rc_cur[:, w * S * F:(w + 1) * S * F],
                                rc_cur[:, w * S * F:(w + 1) * S * F],
                                LRC[:, w * S * F:(w + 1) * S * F],
                                0.0, ALU.max, ALU.add)
                    else:
                        # t[j] = max(u_prev[j-1], u_prev[j] * (j-1)/j)
                        # prev cell j' at slot j'-k+3; j=k+1..16:
                        #   u_prev[j]   -> prev slots 4..S_prev-1... wait
                        #   u_prev[j]   -> prev slots j-k+3 = 4..S_prev+... 
                        # (S_prev = 19-k): u_prev[j] slots 4..18-k? No:
                        # j=k+1 -> slot 4?? j-(k-1)+1 with data base 2:
                        # prev cell j' maps to slot j'-(k-1)+1 = j'-k+2.
                        # u_prev[j], j=k+1..16 -> prev slots 3..18-k
                        # u_prev[j-1], j-1=k..15 -> prev slots 2..17-k
                        for w in range(2):
                            nc.vector.tensor_tensor(
                                reg_fs(rc_cur, k, w)[:, :, 2:S],
                                reg_fs(rc_prev, k - 1, w)[:, :, 3:S + 1],
                                rbigT[:, :, k + 1:17], ALU.mult)
                            nc.vector.tensor_tensor(
                                reg(rc_cur, k, w)[:, 2:S, :],
                                reg(rc_cur, k, w)[:, 2:S, :],
                                reg(rc_prev, k - 1, w)[:, 2:S, :], ALU.max)
                            nc.vector.tensor_tensor_scan(
                                rc_cur[:, w * S * F:(w + 1) * S * F],
                                rc_cur[:, w * S * F:(w + 1) * S * F],
                                LRC[:, w * S * F:(w + 1) * S * F],
                                0.0, ALU.max, ALU.add)

                    rc_prev = rc_cur

            out_sb = mp.tile([P, F], DT)
            nc.vector.tensor_scalar_mul(out_sb[:, :], ud[T % 2][:, :], 1.0 / T)
            nc.sync.dma_start(out_d.ap(), out_sb[:, :])

    nc.compile()
    return nc


def _consts():
    ident = np.eye(128, dtype=np.float32)
    rb = np.zeros((17, F), dtype=np.float32)
    for m in range(1, 17):
        rb[m, :] = (m - 1) / m
    rbT = np.ascontiguousarray(np.broadcast_to(rb[:, 0], (F, 17)))
    rbigT = np.broadcast_to(rbT.reshape(1, F * 17), (P, F * 17)).copy()
    return ident, rbigT


def kernel(a: np.ndarray, b: np.ndarray) -> np.ndarray:
    a = np.ascontiguousarray(a, dtype=np.float32)
    b = np.ascontiguousarray(b, dtype=np.float32)
    assert a.shape == (NA, T, D) and b.shape == (NB, T, D)

    nc = build_program()
    ident, rbigT = _consts()

    in_maps = []
    for core in range(8):
        ca, cb = core // 2, core % 2
        in_maps.append({
            "a_c": a[ca * ACH:(ca + 1) * ACH],
            "b_c": b[cb * BCH:(cb + 1) * BCH],
            "ident": ident,
            "rbigT": rbigT,
        })

    res = bass_utils.run_bass_kernel_spmd(nc, in_maps, core_ids=list(range(8)))
    global _last_results
    _last_results = res

    out = np.zeros((NA, NB), dtype=np.float32)
    for core in range(8):
        ca, cb = core // 2, core % 2
        out[ca * ACH:(ca + 1) * ACH, cb * BCH:(cb + 1) * BCH] = \
            res.results[core]["out"]
    return out


# revision 21
# speedup vs baseline: 1.2999x; 1.1845x over previous
"""Trainium2 Bass kernel for nn_DynamicMaxSimilarity — scan-based dual-sweep DP.

Full inputs a,b: [512, 16, 256] f32.
  an = l2norm(tanh(a)) rows; bn likewise
  sim[a,b,i,j] = dot(an[a,i], bn[b,j]);  out[a,b] = DTW-like max-avg DP.

Sharding: 8 cores as 4 a-chunks (128) x 2 b-chunks (256). Per-core block
[128 a, 256 b]; pairs live as [128 partitions (a), 256 free (b)].

DP in the scaled domain u[i,j] = si[i,j]*max(i,j):
  step k: diag (k,k); row sweep cells (k, j>k); col sweep cells (i>k, k).
  Row sweep, per cell j: u = max(t[j], u_left) + lc, with
    t[j] = max(u_prev[j-1], u_prev[j]*(j-1)/j)   (prep, bulk)
  done by ONE tensor_tensor_scan (op0=max, op1=add) over free axis
  (pair-major, slots [inject, data...]): the inject slot (d0=diag+BIG,
  d1=-BIG) resets and seeds the per-pair chain. Col sweep symmetric.
Validated exactly vs the reference recurrence in fp64.
"""

import numpy as np

import concourse.bass as bass
from concourse import bacc
import concourse.mybir as mybir
from concourse.tile import TileContext
from concourse import bass_utils

NA, NB, T, D = 512, 512, 16, 256
ACH, BCH = 128, 256
P = 128
F = BCH              # pairs per partition
KH = D // 128
DT = mybir.dt.float32
F32R = mybir.dt.float32r
ALU = mybir.AluOpType
ACTF = mybir.ActivationFunctionType
BIG = 64.0

_last_results = None


def _normalize_block(nc, pool, wp, x_sb, nt, name):
    # per-quarter pipeline: tanh/sumsq chunks overlap the DMA tail and
    # each other across ACT/DVE
    ssq = pool.tile([P, nt], DT, name=f"{name}_ssq")
    nrm = pool.tile([P, nt], DT, name=f"{name}_nrm")
    rinv = pool.tile([P, nt], DT, name=f"{name}_rinv")
    h = nt // 4
    for q in range(4):
        sl = slice(q * h, (q + 1) * h)
        nc.scalar.activation(x_sb[:, sl, :], x_sb[:, sl, :], ACTF.Tanh)
        sq = wp.tile([P, h, D], DT, name=f"{name}_sq{q}", tag="sq_scr")
        nc.vector.tensor_tensor(sq[:, :, :], x_sb[:, sl, :], x_sb[:, sl, :],
                                ALU.mult)
        nc.vector.tensor_reduce(ssq[:, sl], sq[:, :, :], mybir.AxisListType.X,
                                ALU.add)
        # per-quarter norm finish so early quarters' scales don't wait on
        # the last quarter's sumsq
        nc.scalar.activation(nrm[:, sl], ssq[:, sl], ACTF.Sqrt)
        nc.vector.reciprocal(rinv[:, sl], nrm[:, sl])
        for i in range(q * h, (q + 1) * h):
            nc.vector.tensor_scalar_mul(x_sb[:, i, :], x_sb[:, i, :],
                                        rinv[:, i:i + 1])
    return x_sb


def build_program():
    nc = bacc.Bacc("TRN2", target_bir_lowering=False, debug=False)

    a_d = nc.dram_tensor("a_c", [ACH, T, D], DT, kind="ExternalInput")
    b_d = nc.dram_tensor("b_c", [BCH, T, D], DT, kind="ExternalInput")
    ident_d = nc.dram_tensor("ident", [128, 128], DT, kind="ExternalInput")
    rbigT_d = nc.dram_tensor("rbigT", [P, 17 * F], DT, kind="ExternalInput")
    out_d = nc.dram_tensor("out", [ACH, BCH], DT, kind="ExternalOutput")

    with TileContext(nc) as tc:
        with (
            tc.tile_pool(name="mp", bufs=1) as mp,
            tc.tile_pool(name="wp", bufs=2) as wp,
        ):
            ident = mp.tile([128, 128], DT)
            nc.sync.dma_start(ident[:, :], ident_d.ap())
            # rbig[m*F + p] = (m-1)/m  (slot-major, constant across pairs)
            rbigT = mp.tile([P, F, 17], DT)
            nc.sync.dma_start(rbigT[:, :, :],
                              rbigT_d.ap().rearrange("p (f m) -> p f m", m=17))

            # ---- load + normalize (in place) ----
            a_sb = mp.tile([P, T, D], DT, tag="sh_lrow")
            for q in range(4):
                nc.sync.dma_start(a_sb[:, q * 4:(q + 1) * 4, :],
                                  a_d.ap()[:, q * 4:(q + 1) * 4, :])
            an = _normalize_block(nc, mp, wp, a_sb, T, "a")

            b_sb = [mp.tile([P, T, D], DT, name=f"b_sb{h}", tag=f"sh_b{h}")
                    for h in range(2)]
            for h in range(2):
                for q in range(4):
                    nc.sync.dma_start(
                        b_sb[h][:, q * 4:(q + 1) * 4, :],
                        b_d.ap()[h * 128:(h + 1) * 128, q * 4:(q + 1) * 4, :])
            bn = [_normalize_block(nc, mp, wp, b_sb[h], T, f"b{h}")
                  for h in range(2)]

            # ---- transposes into matmul layout ----
            anT = mp.tile([P, T, 2, 128], F32R)
            bnT = [mp.tile([P, T, BCH], F32R, name=f"bnT{kh}") for kh in range(KH)]
            with tc.tile_pool(name="tpp", bufs=4, space="PSUM") as tpp:
                def tp_a_group(i0):
                    # 4 transposes (2 i x 2 kh) per PSUM bank, 1 evict
                    ps = tpp.tile([128, 512], DT, name=f"tpa{i0}", tag="tpa",
                                  bufs=2)
                    for q, (i, kh) in enumerate(
                            (i0 + di, kh) for di in range(2) for kh in range(KH)):
                        nc.tensor.transpose(ps[:, q * 128:(q + 1) * 128],
                                            an[:, i, kh * 128:(kh + 1) * 128],
                                            ident[:, :])
                    nc.scalar.activation(
                        anT[:, i0:i0 + 2, :, :],
                        ps[:, :].rearrange("p (i k a) -> p i k a", i=2, k=2),
                        ACTF.Copy)

                # frame 0+1 a-group first (gates step 1's row piece), then
                # all b transposes, then the remaining a-groups (only gate
                # the later col pieces).
                tp_a_group(0)
                for j in range(T):
                    for kh in range(KH):
                        ps = tpp.tile([128, 256], DT, name=f"tpb{j}{kh}",
                                      tag="tpb", bufs=6)
                        for bh in range(2):
                            nc.tensor.transpose(
                                ps[:, bh * 128:(bh + 1) * 128],
                                bn[bh][:, j, kh * 128:(kh + 1) * 128],
                                ident[:, :])
                        if j % 2 == 0:
                            nc.scalar.activation(bnT[kh][:, j, :], ps[:, :],
                                                 ACTF.Copy)
                        else:
                            nc.vector.tensor_copy(bnT[kh][:, j, :], ps[:, :])
                for i0 in range(2, T, 2):
                    tp_a_group(i0)

            # ---- DP tiles ----
            # Row and col sweep state are PACKED in one buffer per step:
            # [row region | col region], each S*F (S=17-k, per-pair slots
            # [inject, data...]), scanned per region (a merged scan acts as
            # a barrier and costs ~5us). a_sb/b_sb memory reused by tags.
            # per-pair slots: [D1, D2, data j=k+1..16]; Sl = 18-k.
            # D1 (d0=udp+BIG, d1=-BIG) resets state to the previous diag;
            # D2 (d0=max(u[k-1,k],u[k,k-1])*(k-1)/k, d1=lc[k,k]) computes the
            # step-k diag INSIDE the scan; its output slot is next step's udp.
            RC0 = mp.tile([P, 2 * 18 * F], DT, tag="sh_lrow")
            RC1 = mp.tile([P, 2 * 18 * F], DT, tag="sh_b0")
            LRC = mp.tile([P, 2 * 18 * F], DT, tag="sh_b1")
            ud = [mp.tile([P, F], DT, name=f"ud{x}") for x in range(2)]

            def reg(tile, k, w):
                """Region w (0=row, 1=col) as [P, slots, pairs] (s, f)."""
                S = 18 - k
                return tile[:, w * S * F:(w + 1) * S * F].rearrange(
                    "p (f s) -> p s f", s=S)

            def reg_fs(tile, k, w):
                """Region w as [P, pairs, slots] (slots contiguous)."""
                S = 18 - k
                return tile[:, w * S * F:(w + 1) * S * F].rearrange(
                    "p (f s) -> p f s", s=S)

            def both(tile, k):
                """[P, region, pairs, slots] 4D view over both regions."""
                S = 18 - k
                return tile[:, 0:2 * S * F].rearrange(
                    "p (w f s) -> p w f s", w=2, s=S)

            rc_prev = None

            # ---- fused matmul + dual-sweep DP ----
            with tc.tile_pool(name="pp", bufs=2, space="PSUM") as pp:
                def mm_planes(psv, i0, j0, np_):
                    """matmul lc planes (frame i0; b-frames j0..j0+np_) into
                    psv [128, np_*F]."""
                    for q0 in range(0, np_, 2):
                        w = min(2, np_ - q0)
                        for kh in range(KH):
                            nc.tensor.matmul(
                                psv[:, q0 * F:(q0 + w) * F],
                                anT[:, i0, kh, :],
                                bnT[kh][:, j0 + q0:j0 + q0 + w, :],
                                start=(kh == 0), stop=(kh == KH - 1),
                            )

                for k in range(1, T + 1):
                    S = 18 - k   # slots per pair: [D1, D2, data...]
                    udp, udc = ud[(k - 1) % 2], ud[k % 2]

                    # --- produce lc L-border for step k ---
                    # diag+row piece: frame k-1 x b-frames k-1..15
                    nrow = 17 - k   # includes diag plane
                    row_chunks = []
                    for c0 in range(0, nrow, 4):
                        w = min(4, nrow - c0)
                        psv = pp.tile([128, 4 * F], DT,
                                      name=f"psr{k}_{c0}", tag="locr")
                        mm_planes(psv, k - 1, (k - 1) + c0, w)
                        row_chunks.append((c0, w, psv))
                    # col piece: frames k..15 x b-frame k-1
                    ncol = 16 - k
                    col_chunks = []
                    for c0 in range(0, ncol, 4):
                        w = min(4, ncol - c0)
                        psv = pp.tile([128, 4 * F], DT,
                                      name=f"psc{k}_{c0}", tag="locc")
                        for q in range(w):
                            for kh in range(KH):
                                nc.tensor.matmul(
                                    psv[:, q * F:(q + 1) * F],
                                    anT[:, k + c0 + q, kh, :],
                                    bnT[kh][:, k - 1:k, :],
                                    start=(kh == 0), stop=(kh == KH - 1),
                                )
                        col_chunks.append((c0, w, psv))

                    # --- diag cell (k,k) ---
                    lc_diag = row_chunks[0][2][:, 0:F]
                    m1 = None
                    if k > 1:
                        m1 = wp.tile([P, F], DT, name=f"m1_{k}", tag="m1")
                        nc.vector.tensor_tensor(
                            m1[:, :], reg(rc_prev, k - 1, 0)[:, 2, :],
                            reg(rc_prev, k - 1, 1)[:, 2, :], ALU.max)
                    if k == T:
                        # no scans at k=16: explicit diag
                        udpT = reg(rc_prev, k - 1, 0)[:, 1, :]
                        nc.vector.scalar_tensor_tensor(
                            m1[:, :], m1[:, :], float((k - 1) / k), udpT,
                            ALU.mult, ALU.max)
                        nc.vector.tensor_tensor(udc[:, :], m1[:, :], lc_diag,
                                                ALU.add)
                        break

                    # --- sweeps ---
                    rc_cur = RC0 if (k % 2) else RC1
                    tr_cur = reg(rc_cur, k, 0)
                    tc_cur = reg(rc_cur, k, 1)
                    tl_row = reg(LRC, k, 0)
                    tl_col = reg(LRC, k, 1)

                    # injects. D1: d0 = udp + BIG (prev diag from the prev
                    # row scan's D2 slot), d1 = -BIG. D2: d0 = m1*(k-1)/k
                    # (ACT scaled copy), d1 = lc_diag (rides the evictions).
                    if k == 1:
                        nc.gpsimd.memset(both(rc_cur, k)[:, :, :, 0], BIG)
                    else:
                        udp_sl = reg(rc_prev, k - 1, 0)[:, 1, :]
                        nc.scalar.activation(tr_cur[:, 0, :], udp_sl,
                                             ACTF.Copy, bias=BIG)
                        nc.scalar.activation(tc_cur[:, 0, :], udp_sl,
                                             ACTF.Copy, bias=BIG)
                        nc.scalar.activation(tr_cur[:, 1, :], m1[:, :],
                                             ACTF.Copy,
                                             scale=float((k - 1) / k))
                        nc.scalar.activation(tc_cur[:, 1, :], m1[:, :],
                                             ACTF.Copy,
                                             scale=float((k - 1) / k))
                    nc.gpsimd.memset(both(LRC, k)[:, :, :, 0], -BIG)
                    # col region's D2 d1 = lc_diag (row region's comes with
                    # the row chunk eviction below)
                    nc.scalar.activation(tl_col[:, 1, :], lc_diag, ACTF.Copy)

                    # evictions into d1 slots: b-frame plane p lands at
                    # slot p-k+2 (the diag plane p=k-1 lands at D2=slot 1).
                    for (c0, w, psv) in row_chunks:
                        nc.scalar.activation(
                            tl_row[:, 1 + c0:1 + c0 + w, :],
                            psv[:, 0:w * F].rearrange("p (n f) -> p n f", f=F),
                            ACTF.Copy)
                    for (c0, w, psv) in col_chunks:
                        nc.scalar.activation(
                            tl_col[:, 2 + c0:2 + c0 + w, :],
                            psv[:, 0:w * F].rearrange("p (n f) -> p n f", f=F),
                            ACTF.Copy)

                    # prep t data slots (data at slots 2..S-1)
                    if k == 1:
                        # D2 + data slots = 0 (disjoint from the D1 writes)
                        nc.gpsimd.memset(both(rc_cur, k)[:, :, :, 1:S], 0.0)
                        for w in range(2):
                            nc.vector.tensor_tensor_scan(
                                rc_cur[:, w * S * F:(w + 1) * S * F],
                                rc_cur[:, w * S * F:(w + 1) * S * F],
                                LRC[:, w * S * F:(w + 1) * S * F],
                                0.0, ALU.max, ALU.add)
                    else:
                        # t[j] = max(u_prev[j-1], u_prev[j] * (j-1)/j).
                        # (j-1)/j is CONSTANT per slot, so one per-slot
                        # scalar_tensor_tensor (imm scalar) fuses the mult
                        # and the max into a single streaming pass — half
                        # the element visits of the bulk mult+max pair.
                        # cur cell j at slot c=j-k+1; prev cell j' at slot
                        # j'-k+2 -> in0 = prev[c+1] (u_prev[j]),
                        # in1 = prev[c] (u_prev[j-1]).
                        for w in range(2):
                            rpv = reg(rc_prev, k - 1, w)
                            rcv = reg(rc_cur, k, w)
                            for c in range(2, S):
                                j = k + c - 1
                                nc.vector.scalar_tensor_tensor(
                                    rcv[:, c, :], rpv[:, c + 1, :],
                                    float((j - 1) / j), rpv[:, c, :],
                                    ALU.mult, ALU.max)
                            nc.vector.tensor_tensor_scan(
                                rc_cur[:, w * S * F:(w + 1) * S * F],
                                rc_cur[:, w * S * F:(w + 1) * S * F],
                                LRC[:, w * S * F:(w + 1) * S * F],
                                0.0, ALU.max, ALU.add)

                    rc_prev = rc_cur

            out_sb = mp.tile([P, F], DT)
            nc.vector.tensor_scalar_mul(out_sb[:, :], ud[T % 2][:, :], 1.0 / T)
            nc.sync.dma_start(out_d.ap(), out_sb[:, :])

    nc.compile()
    return nc


def _consts():
    ident = np.eye(128, dtype=np.float32)
    rb = np.zeros((17, F), dtype=np.float32)
    for m in range(1, 17):
        rb[m, :] = (m - 1) / m
    rbT = np.ascontiguousarray(np.broadcast_to(rb[:, 0], (F, 17)))
    rbigT = np.broadcast_to(rbT.reshape(1, F * 17), (P, F * 17)).copy()
    return ident, rbigT


def kernel(a: np.ndarray, b: np.ndarray) -> np.ndarray:
    a = np.ascontiguousarray(a, dtype=np.float32)
    b = np.ascontiguousarray(b, dtype=np.float32)
    assert a.shape == (NA, T, D) and b.shape == (NB, T, D)

    nc = build_program()
    ident, rbigT = _consts()

    in_maps = []
    for core in range(8):
        ca, cb = core // 2, core % 2
        in_maps.append({
            "a_c": a[ca * ACH:(ca + 1) * ACH],
            "b_c": b[cb * BCH:(cb + 1) * BCH],
            "ident": ident,
            "rbigT": rbigT,
        })

    res = bass_utils.run_bass_kernel_spmd(nc, in_maps, core_ids=list(range(8)))
    global _last_results
    _last_results = res

    out = np.zeros((NA, NB), dtype=np.float32)
    for core in range(8):
        ca, cb = core // 2, core % 2
        out[ca * ACH:(ca + 1) * ACH, cb * BCH:(cb + 1) * BCH] = \
            res.results[core]["out"]
    return out


# revision 22
# speedup vs baseline: 1.3357x; 1.0275x over previous
"""Trainium2 Bass kernel for nn_DynamicMaxSimilarity — scan-based dual-sweep DP.

Full inputs a,b: [512, 16, 256] f32.
  an = l2norm(tanh(a)) rows; bn likewise
  sim[a,b,i,j] = dot(an[a,i], bn[b,j]);  out[a,b] = DTW-like max-avg DP.

Sharding: 8 cores as 4 a-chunks (128) x 2 b-chunks (256). Per-core block
[128 a, 256 b]; pairs live as [128 partitions (a), 256 free (b)].

DP in the scaled domain u[i,j] = si[i,j]*max(i,j):
  step k: diag (k,k); row sweep cells (k, j>k); col sweep cells (i>k, k).
  Row sweep, per cell j: u = max(t[j], u_left) + lc, with
    t[j] = max(u_prev[j-1], u_prev[j]*(j-1)/j)   (prep, bulk)
  done by ONE tensor_tensor_scan (op0=max, op1=add) over free axis
  (pair-major, slots [inject, data...]): the inject slot (d0=diag+BIG,
  d1=-BIG) resets and seeds the per-pair chain. Col sweep symmetric.
Validated exactly vs the reference recurrence in fp64.
"""

import numpy as np

import concourse.bass as bass
from concourse import bacc
import concourse.mybir as mybir
from concourse.tile import TileContext
from concourse import bass_utils

NA, NB, T, D = 512, 512, 16, 256
ACH, BCH = 128, 256
P = 128
F = BCH              # pairs per partition
KH = D // 128
DT = mybir.dt.float32
F32R = mybir.dt.float32r
ALU = mybir.AluOpType
ACTF = mybir.ActivationFunctionType
BIG = 64.0

_last_results = None


def _normalize_block(nc, pool, wp, x_sb, nt, name):
    # per-quarter pipeline: tanh/sumsq chunks overlap the DMA tail and
    # each other across ACT/DVE
    ssq = pool.tile([P, nt], DT, name=f"{name}_ssq")
    nrm = pool.tile([P, nt], DT, name=f"{name}_nrm")
    rinv = pool.tile([P, nt], DT, name=f"{name}_rinv")
    h = nt // 4
    for q in range(4):
        sl = slice(q * h, (q + 1) * h)
        nc.scalar.activation(x_sb[:, sl, :], x_sb[:, sl, :], ACTF.Tanh)
        sq = wp.tile([P, h, D], DT, name=f"{name}_sq{q}", tag="sq_scr")
        nc.vector.tensor_tensor(sq[:, :, :], x_sb[:, sl, :], x_sb[:, sl, :],
                                ALU.mult)
        nc.vector.tensor_reduce(ssq[:, sl], sq[:, :, :], mybir.AxisListType.X,
                                ALU.add)
        # per-quarter norm finish so early quarters' scales don't wait on
        # the last quarter's sumsq
        nc.scalar.activation(nrm[:, sl], ssq[:, sl], ACTF.Sqrt)
        nc.vector.reciprocal(rinv[:, sl], nrm[:, sl])
        for i in range(q * h, (q + 1) * h):
            nc.vector.tensor_scalar_mul(x_sb[:, i, :], x_sb[:, i, :],
                                        rinv[:, i:i + 1])
    return x_sb


def build_program():
    nc = bacc.Bacc("TRN2", target_bir_lowering=False, debug=False)

    a_d = nc.dram_tensor("a_c", [ACH, T, D], DT, kind="ExternalInput")
    b_d = nc.dram_tensor("b_c", [BCH, T, D], DT, kind="ExternalInput")
    ident_d = nc.dram_tensor("ident", [128, 128], DT, kind="ExternalInput")
    out_d = nc.dram_tensor("out", [ACH, BCH], DT, kind="ExternalOutput")

    with TileContext(nc) as tc:
        with (
            tc.tile_pool(name="mp", bufs=1) as mp,
            tc.tile_pool(name="wp", bufs=2) as wp,
        ):
            ident = mp.tile([128, 128], DT)
            nc.sync.dma_start(ident[:, :], ident_d.ap())

            # ---- load + normalize (in place) ----
            a_sb = mp.tile([P, T, D], DT, tag="sh_lrow")
            for q in range(4):
                nc.sync.dma_start(a_sb[:, q * 4:(q + 1) * 4, :],
                                  a_d.ap()[:, q * 4:(q + 1) * 4, :])
            an = _normalize_block(nc, mp, wp, a_sb, T, "a")

            b_sb = [mp.tile([P, T, D], DT, name=f"b_sb{h}", tag=f"sh_b{h}")
                    for h in range(2)]
            for h in range(2):
                for q in range(4):
                    nc.sync.dma_start(
                        b_sb[h][:, q * 4:(q + 1) * 4, :],
                        b_d.ap()[h * 128:(h + 1) * 128, q * 4:(q + 1) * 4, :])
            bn = [_normalize_block(nc, mp, wp, b_sb[h], T, f"b{h}")
                  for h in range(2)]

            # ---- transposes into matmul layout ----
            anT = mp.tile([P, T, 2, 128], F32R)
            bnT = [mp.tile([P, T, BCH], F32R, name=f"bnT{kh}") for kh in range(KH)]
            with tc.tile_pool(name="tpp", bufs=4, space="PSUM") as tpp:
                def tp_a_group(i0):
                    # 4 transposes (2 i x 2 kh) per PSUM bank, 1 evict
                    ps = tpp.tile([128, 512], DT, name=f"tpa{i0}", tag="tpa",
                                  bufs=2)
                    for q, (i, kh) in enumerate(
                            (i0 + di, kh) for di in range(2) for kh in range(KH)):
                        nc.tensor.transpose(ps[:, q * 128:(q + 1) * 128],
                                            an[:, i, kh * 128:(kh + 1) * 128],
                                            ident[:, :])
                    nc.scalar.activation(
                        anT[:, i0:i0 + 2, :, :],
                        ps[:, :].rearrange("p (i k a) -> p i k a", i=2, k=2),
                        ACTF.Copy)

                # frame 0+1 a-group first (gates step 1's row piece), then
                # all b transposes, then the remaining a-groups (only gate
                # the later col pieces).
                tp_a_group(0)
                for j in range(T):
                    for kh in range(KH):
                        ps = tpp.tile([128, 256], DT, name=f"tpb{j}{kh}",
                                      tag="tpb", bufs=6)
                        for bh in range(2):
                            nc.tensor.transpose(
                                ps[:, bh * 128:(bh + 1) * 128],
                                bn[bh][:, j, kh * 128:(kh + 1) * 128],
                                ident[:, :])
                        if j % 2 == 0:
                            nc.scalar.activation(bnT[kh][:, j, :], ps[:, :],
                                                 ACTF.Copy)
                        else:
                            nc.vector.tensor_copy(bnT[kh][:, j, :], ps[:, :])
                for i0 in range(2, T, 2):
                    tp_a_group(i0)

            # ---- DP tiles ----
            # Row and col sweep state are PACKED in one buffer per step:
            # [row region | col region], each S*F (S=17-k, per-pair slots
            # [inject, data...]), scanned per region (a merged scan acts as
            # a barrier and costs ~5us). a_sb/b_sb memory reused by tags.
            # per-pair slots: [D1, D2, data j=k+1..16]; Sl = 18-k.
            # D1 (d0=udp+BIG, d1=-BIG) resets state to the previous diag;
            # D2 (d0=max(u[k-1,k],u[k,k-1])*(k-1)/k, d1=lc[k,k]) computes the
            # step-k diag INSIDE the scan; its output slot is next step's udp.
            RC0 = mp.tile([P, 2 * 18 * F], DT, tag="sh_lrow")
            RC1 = mp.tile([P, 2 * 18 * F], DT, tag="sh_b0")
            LRC = mp.tile([P, 2 * 18 * F], DT, tag="sh_b1")
            ud = [mp.tile([P, F], DT, name=f"ud{x}") for x in range(2)]

            def reg(tile, k, w):
                """Region w (0=row, 1=col) as [P, slots, pairs] (s, f)."""
                S = 18 - k
                return tile[:, w * S * F:(w + 1) * S * F].rearrange(
                    "p (f s) -> p s f", s=S)

            def reg_fs(tile, k, w):
                """Region w as [P, pairs, slots] (slots contiguous)."""
                S = 18 - k
                return tile[:, w * S * F:(w + 1) * S * F].rearrange(
                    "p (f s) -> p f s", s=S)

            def both(tile, k):
                """[P, region, pairs, slots] 4D view over both regions."""
                S = 18 - k
                return tile[:, 0:2 * S * F].rearrange(
                    "p (w f s) -> p w f s", w=2, s=S)

            rc_prev = None

            # ---- fused matmul + dual-sweep DP ----
            with tc.tile_pool(name="pp", bufs=2, space="PSUM") as pp:
                def mm_planes(psv, i0, j0, np_):
                    """matmul lc planes (frame i0; b-frames j0..j0+np_) into
                    psv [128, np_*F]."""
                    for q0 in range(0, np_, 2):
                        w = min(2, np_ - q0)
                        for kh in range(KH):
                            nc.tensor.matmul(
                                psv[:, q0 * F:(q0 + w) * F],
                                anT[:, i0, kh, :],
                                bnT[kh][:, j0 + q0:j0 + q0 + w, :],
                                start=(kh == 0), stop=(kh == KH - 1),
                            )

                for k in range(1, T + 1):
                    S = 18 - k   # slots per pair: [D1, D2, data...]
                    udp, udc = ud[(k - 1) % 2], ud[k % 2]

                    # --- produce lc L-border for step k ---
                    # diag+row piece: frame k-1 x b-frames k-1..15
                    nrow = 17 - k   # includes diag plane
                    row_chunks = []
                    for c0 in range(0, nrow, 4):
                        w = min(4, nrow - c0)
                        psv = pp.tile([128, 4 * F], DT,
                                      name=f"psr{k}_{c0}", tag="locr")
                        mm_planes(psv, k - 1, (k - 1) + c0, w)
                        row_chunks.append((c0, w, psv))
                    # col piece: frames k..15 x b-frame k-1
                    ncol = 16 - k
                    col_chunks = []
                    for c0 in range(0, ncol, 4):
                        w = min(4, ncol - c0)
                        psv = pp.tile([128, 4 * F], DT,
                                      name=f"psc{k}_{c0}", tag="locc")
                        for q in range(w):
                            for kh in range(KH):
                                nc.tensor.matmul(
                                    psv[:, q * F:(q + 1) * F],
                                    anT[:, k + c0 + q, kh, :],
                                    bnT[kh][:, k - 1:k, :],
                                    start=(kh == 0), stop=(kh == KH - 1),
                                )
                        col_chunks.append((c0, w, psv))

                    # --- diag cell (k,k) ---
                    lc_diag = row_chunks[0][2][:, 0:F]
                    m1 = None
                    if k > 1:
                        m1 = wp.tile([P, F], DT, name=f"m1_{k}", tag="m1")
                        nc.vector.tensor_tensor(
                            m1[:, :], reg(rc_prev, k - 1, 0)[:, 2, :],
                            reg(rc_prev, k - 1, 1)[:, 2, :], ALU.max)
                    if k == T:
                        # no scans at k=16: explicit diag
                        udpT = reg(rc_prev, k - 1, 0)[:, 1, :]
                        nc.vector.scalar_tensor_tensor(
                            m1[:, :], m1[:, :], float((k - 1) / k), udpT,
                            ALU.mult, ALU.max)
                        nc.vector.tensor_tensor(udc[:, :], m1[:, :], lc_diag,
                                                ALU.add)
                        break

                    # --- sweeps ---
                    rc_cur = RC0 if (k % 2) else RC1
                    tr_cur = reg(rc_cur, k, 0)
                    tc_cur = reg(rc_cur, k, 1)
                    tl_row = reg(LRC, k, 0)
                    tl_col = reg(LRC, k, 1)

                    # injects. D1: d0 = udp + BIG (prev diag from the prev
                    # row scan's D2 slot), d1 = -BIG. D2: d0 = m1*(k-1)/k
                    # (ACT scaled copy), d1 = lc_diag (rides the evictions).
                    if k == 1:
                        nc.gpsimd.memset(both(rc_cur, k)[:, :, :, 0], BIG)
                    else:
                        udp_sl = reg(rc_prev, k - 1, 0)[:, 1, :]
                        nc.scalar.activation(tr_cur[:, 0, :], udp_sl,
                                             ACTF.Copy, bias=BIG)
                        nc.scalar.activation(tc_cur[:, 0, :], udp_sl,
                                             ACTF.Copy, bias=BIG)
                        nc.scalar.activation(tr_cur[:, 1, :], m1[:, :],
                                             ACTF.Copy,
                                             scale=float((k - 1) / k))
                        nc.scalar.activation(tc_cur[:, 1, :], m1[:, :],
                                             ACTF.Copy,
                                             scale=float((k - 1) / k))
                    nc.gpsimd.memset(both(LRC, k)[:, :, :, 0], -BIG)
                    # col region's D2 d1 = lc_diag (row region's comes with
                    # the row chunk eviction below)
                    nc.scalar.activation(tl_col[:, 1, :], lc_diag, ACTF.Copy)

                    # evictions into d1 slots: b-frame plane p lands at
                    # slot p-k+2 (the diag plane p=k-1 lands at D2=slot 1).
                    for (c0, w, psv) in row_chunks:
                        nc.scalar.activation(
                            tl_row[:, 1 + c0:1 + c0 + w, :],
                            psv[:, 0:w * F].rearrange("p (n f) -> p n f", f=F),
                            ACTF.Copy)
                    for (c0, w, psv) in col_chunks:
                        nc.scalar.activation(
                            tl_col[:, 2 + c0:2 + c0 + w, :],
                            psv[:, 0:w * F].rearrange("p (n f) -> p n f", f=F),
                            ACTF.Copy)

                    # prep t data slots (data at slots 2..S-1)
                    if k == 1:
                        # D2 + data slots = 0 (disjoint from the D1 writes)
                        nc.gpsimd.memset(both(rc_cur, k)[:, :, :, 1:S], 0.0)
                        for w in range(2):
                            nc.vector.tensor_tensor_scan(
                                rc_cur[:, w * S * F:(w + 1) * S * F],
                                rc_cur[:, w * S * F:(w + 1) * S * F],
                                LRC[:, w * S * F:(w + 1) * S * F],
                                0.0, ALU.max, ALU.add)
                    else:
                        # t[j] = max(u_prev[j-1], u_prev[j] * (j-1)/j).
                        # (j-1)/j is CONSTANT per slot, so one per-slot
                        # scalar_tensor_tensor (imm scalar) fuses the mult
                        # and the max into a single streaming pass — half
                        # the element visits of the bulk mult+max pair.
                        # cur cell j at slot c=j-k+1; prev cell j' at slot
                        # j'-k+2 -> in0 = prev[c+1] (u_prev[j]),
                        # in1 = prev[c] (u_prev[j-1]).
                        for w in range(2):
                            rpv = reg(rc_prev, k - 1, w)
                            rcv = reg(rc_cur, k, w)
                            for c in range(2, S):
                                j = k + c - 1
                                nc.vector.scalar_tensor_tensor(
                                    rcv[:, c, :], rpv[:, c + 1, :],
                                    float((j - 1) / j), rpv[:, c, :],
                                    ALU.mult, ALU.max)
                            nc.vector.tensor_tensor_scan(
                                rc_cur[:, w * S * F:(w + 1) * S * F],
                                rc_cur[:, w * S * F:(w + 1) * S * F],
                                LRC[:, w * S * F:(w + 1) * S * F],
                                0.0, ALU.max, ALU.add)

                    rc_prev = rc_cur

            out_sb = mp.tile([P, F], DT)
            nc.vector.tensor_scalar_mul(out_sb[:, :], ud[T % 2][:, :], 1.0 / T)
            nc.sync.dma_start(out_d.ap(), out_sb[:, :])

    nc.compile()
    return nc


def _consts():
    ident = np.eye(128, dtype=np.float32)
    return ident


def kernel(a: np.ndarray, b: np.ndarray) -> np.ndarray:
    a = np.ascontiguousarray(a, dtype=np.float32)
    b = np.ascontiguousarray(b, dtype=np.float32)
    assert a.shape == (NA, T, D) and b.shape == (NB, T, D)

    nc = build_program()
    ident = _consts()

    in_maps = []
    for core in range(8):
        ca, cb = core // 2, core % 2
        in_maps.append({
            "a_c": a[ca * ACH:(ca + 1) * ACH],
            "b_c": b[cb * BCH:(cb + 1) * BCH],
            "ident": ident,
        })

    res = bass_utils.run_bass_kernel_spmd(nc, in_maps, core_ids=list(range(8)))
    global _last_results
    _last_results = res

    out = np.zeros((NA, NB), dtype=np.float32)
    for core in range(8):
        ca, cb = core // 2, core % 2
        out[ca * ACH:(ca + 1) * ACH, cb * BCH:(cb + 1) * BCH] = \
            res.results[core]["out"]
    return out


# revision 23
# speedup vs baseline: 1.3607x; 1.0187x over previous
"""Trainium2 Bass kernel for nn_DynamicMaxSimilarity — scan-based dual-sweep DP.

Full inputs a,b: [512, 16, 256] f32.
  an = l2norm(tanh(a)) rows; bn likewise
  sim[a,b,i,j] = dot(an[a,i], bn[b,j]);  out[a,b] = DTW-like max-avg DP.

Sharding: 8 cores as 4 a-chunks (128) x 2 b-chunks (256). Per-core block
[128 a, 256 b]; pairs live as [128 partitions (a), 256 free (b)].

DP in the scaled domain u[i,j] = si[i,j]*max(i,j):
  step k: diag (k,k); row sweep cells (k, j>k); col sweep cells (i>k, k).
  Row sweep, per cell j: u = max(t[j], u_left) + lc, with
    t[j] = max(u_prev[j-1], u_prev[j]*(j-1)/j)   (prep, bulk)
  done by ONE tensor_tensor_scan (op0=max, op1=add) over free axis
  (pair-major, slots [inject, data...]): the inject slot (d0=diag+BIG,
  d1=-BIG) resets and seeds the per-pair chain. Col sweep symmetric.
Validated exactly vs the reference recurrence in fp64.
"""

import numpy as np

import concourse.bass as bass
from concourse import bacc
import concourse.mybir as mybir
from concourse.tile import TileContext
from concourse import bass_utils

NA, NB, T, D = 512, 512, 16, 256
ACH, BCH = 128, 256
P = 128
F = BCH              # pairs per partition
KH = D // 128
DT = mybir.dt.float32
F32R = mybir.dt.float32r
ALU = mybir.AluOpType
ACTF = mybir.ActivationFunctionType
BIG = 64.0

_last_results = None


def _normalize_block(nc, pool, wp, x_sb, nt, name):
    # per-quarter pipeline: tanh/sumsq chunks overlap the DMA tail and
    # each other across ACT/DVE
    ssq = pool.tile([P, nt], DT, name=f"{name}_ssq")
    nrm = pool.tile([P, nt], DT, name=f"{name}_nrm")
    rinv = pool.tile([P, nt], DT, name=f"{name}_rinv")
    h = nt // 4
    for q in range(4):
        sl = slice(q * h, (q + 1) * h)
        nc.scalar.activation(x_sb[:, sl, :], x_sb[:, sl, :], ACTF.Tanh)
        sq = wp.tile([P, h, D], DT, name=f"{name}_sq{q}", tag="sq_scr")
        nc.vector.tensor_tensor(sq[:, :, :], x_sb[:, sl, :], x_sb[:, sl, :],
                                ALU.mult)
        nc.vector.tensor_reduce(ssq[:, sl], sq[:, :, :], mybir.AxisListType.X,
                                ALU.add)
        # per-quarter norm finish so early quarters' scales don't wait on
        # the last quarter's sumsq
        nc.scalar.activation(nrm[:, sl], ssq[:, sl], ACTF.Sqrt)
        nc.vector.reciprocal(rinv[:, sl], nrm[:, sl])
        for i in range(q * h, (q + 1) * h):
            nc.vector.tensor_scalar_mul(x_sb[:, i, :], x_sb[:, i, :],
                                        rinv[:, i:i + 1])
    return x_sb


def build_program():
    nc = bacc.Bacc("TRN2", target_bir_lowering=False, debug=False)

    a_d = nc.dram_tensor("a_c", [ACH, T, D], DT, kind="ExternalInput")
    b_d = nc.dram_tensor("b_c", [BCH, T, D], DT, kind="ExternalInput")
    ident_d = nc.dram_tensor("ident", [128, 128], DT, kind="ExternalInput")
    out_d = nc.dram_tensor("out", [ACH, BCH], DT, kind="ExternalOutput")

    with TileContext(nc) as tc:
        with (
            tc.tile_pool(name="mp", bufs=1) as mp,
            tc.tile_pool(name="wp", bufs=2) as wp,
        ):
            ident = mp.tile([128, 128], DT)
            nc.sync.dma_start(ident[:, :], ident_d.ap())

            # ---- load + normalize (in place) ----
            a_sb = mp.tile([P, T, D], DT, tag="sh_lrow")
            for q in range(4):
                nc.sync.dma_start(a_sb[:, q * 4:(q + 1) * 4, :],
                                  a_d.ap()[:, q * 4:(q + 1) * 4, :])
            an = _normalize_block(nc, mp, wp, a_sb, T, "a")

            b_sb = [mp.tile([P, T, D], DT, name=f"b_sb{h}", tag=f"sh_b{h}")
                    for h in range(2)]
            for h in range(2):
                for q in range(4):
                    nc.sync.dma_start(
                        b_sb[h][:, q * 4:(q + 1) * 4, :],
                        b_d.ap()[h * 128:(h + 1) * 128, q * 4:(q + 1) * 4, :])
            bn = [_normalize_block(nc, mp, wp, b_sb[h], T, f"b{h}")
                  for h in range(2)]

            # ---- transposes into matmul layout ----
            anT = mp.tile([P, T, 2, 128], F32R)
            bnT = [mp.tile([P, T, BCH], F32R, name=f"bnT{kh}") for kh in range(KH)]
            with tc.tile_pool(name="tpp", bufs=4, space="PSUM") as tpp:
                def tp_a_group(i0):
                    # 4 transposes (2 i x 2 kh) per PSUM bank, 1 evict
                    ps = tpp.tile([128, 512], DT, name=f"tpa{i0}", tag="tpa",
                                  bufs=2)
                    for q, (i, kh) in enumerate(
                            (i0 + di, kh) for di in range(2) for kh in range(KH)):
                        nc.tensor.transpose(ps[:, q * 128:(q + 1) * 128],
                                            an[:, i, kh * 128:(kh + 1) * 128],
                                            ident[:, :])
                    nc.scalar.activation(
                        anT[:, i0:i0 + 2, :, :],
                        ps[:, :].rearrange("p (i k a) -> p i k a", i=2, k=2),
                        ACTF.Copy)

                # frame 0+1 a-group first (gates step 1's row piece), then
                # all b transposes, then the remaining a-groups (only gate
                # the later col pieces).
                tp_a_group(0)
                for j in range(T):
                    for kh in range(KH):
                        ps = tpp.tile([128, 256], DT, name=f"tpb{j}{kh}",
                                      tag="tpb", bufs=6)
                        for bh in range(2):
                            nc.tensor.transpose(
                                ps[:, bh * 128:(bh + 1) * 128],
                                bn[bh][:, j, kh * 128:(kh + 1) * 128],
                                ident[:, :])
                        if j % 2 == 0:
                            nc.scalar.activation(bnT[kh][:, j, :], ps[:, :],
                                                 ACTF.Copy)
                        else:
                            nc.vector.tensor_copy(bnT[kh][:, j, :], ps[:, :])
                for i0 in range(2, T, 2):
                    tp_a_group(i0)

            # ---- DP tiles ----
            # Row and col sweep state are PACKED in one buffer per step:
            # [row region | col region], each S*F (S=17-k, per-pair slots
            # [inject, data...]), scanned per region (a merged scan acts as
            # a barrier and costs ~5us). a_sb/b_sb memory reused by tags.
            # per-pair slots: [D1, D2, data j=k+1..16]; Sl = 18-k.
            # D1 (d0=udp+BIG, d1=-BIG) resets state to the previous diag;
            # D2 (d0=max(u[k-1,k],u[k,k-1])*(k-1)/k, d1=lc[k,k]) computes the
            # step-k diag INSIDE the scan; its output slot is next step's udp.
            RC0 = mp.tile([P, 2 * 18 * F], DT, tag="sh_lrow")
            RC1 = mp.tile([P, 2 * 18 * F], DT, tag="sh_b0")
            LRC = mp.tile([P, 2 * 18 * F], DT, tag="sh_b1")
            ud = [mp.tile([P, F], DT, name=f"ud{x}") for x in range(2)]

            def reg(tile, k, w):
                """Region w (0=row, 1=col) as [P, slots, pairs] (s, f)."""
                S = 18 - k
                return tile[:, w * S * F:(w + 1) * S * F].rearrange(
                    "p (f s) -> p s f", s=S)

            def reg_fs(tile, k, w):
                """Region w as [P, pairs, slots] (slots contiguous)."""
                S = 18 - k
                return tile[:, w * S * F:(w + 1) * S * F].rearrange(
                    "p (f s) -> p f s", s=S)

            def both(tile, k):
                """[P, region, pairs, slots] 4D view over both regions."""
                S = 18 - k
                return tile[:, 0:2 * S * F].rearrange(
                    "p (w f s) -> p w f s", w=2, s=S)

            rc_prev = None

            # ---- fused matmul + dual-sweep DP ----
            with tc.tile_pool(name="pp", bufs=2, space="PSUM") as pp:
                def mm_planes(psv, i0, j0, np_):
                    """matmul lc planes (frame i0; b-frames j0..j0+np_) into
                    psv [128, np_*F]."""
                    for q0 in range(0, np_, 2):
                        w = min(2, np_ - q0)
                        for kh in range(KH):
                            nc.tensor.matmul(
                                psv[:, q0 * F:(q0 + w) * F],
                                anT[:, i0, kh, :],
                                bnT[kh][:, j0 + q0:j0 + q0 + w, :],
                                start=(kh == 0), stop=(kh == KH - 1),
                            )

                for k in range(1, T + 1):
                    S = 18 - k   # slots per pair: [D1, D2, data...]
                    udp, udc = ud[(k - 1) % 2], ud[k % 2]

                    # --- produce lc L-border for step k ---
                    # diag+row piece: frame k-1 x b-frames k-1..15
                    nrow = 17 - k   # includes diag plane
                    row_chunks = []
                    for c0 in range(0, nrow, 4):
                        w = min(4, nrow - c0)
                        psv = pp.tile([128, 4 * F], DT,
                                      name=f"psr{k}_{c0}", tag="locr")
                        mm_planes(psv, k - 1, (k - 1) + c0, w)
                        row_chunks.append((c0, w, psv))
                    # col piece: frames k..15 x b-frame k-1
                    ncol = 16 - k
                    col_chunks = []
                    for c0 in range(0, ncol, 4):
                        w = min(4, ncol - c0)
                        psv = pp.tile([128, 4 * F], DT,
                                      name=f"psc{k}_{c0}", tag="locc")
                        for q in range(w):
                            for kh in range(KH):
                                nc.tensor.matmul(
                                    psv[:, q * F:(q + 1) * F],
                                    anT[:, k + c0 + q, kh, :],
                                    bnT[kh][:, k - 1:k, :],
                                    start=(kh == 0), stop=(kh == KH - 1),
                                )
                        col_chunks.append((c0, w, psv))

                    # --- diag cell (k,k) ---
                    lc_diag = row_chunks[0][2][:, 0:F]
                    m1 = None
                    if k > 1:
                        m1 = wp.tile([P, F], DT, name=f"m1_{k}", tag="m1")
                        nc.vector.tensor_tensor(
                            m1[:, :], reg(rc_prev, k - 1, 0)[:, 2, :],
                            reg(rc_prev, k - 1, 1)[:, 2, :], ALU.max)
                    if k == T:
                        # no scans at k=16: explicit diag
                        udpT = reg(rc_prev, k - 1, 0)[:, 1, :]
                        nc.vector.scalar_tensor_tensor(
                            m1[:, :], m1[:, :], float((k - 1) / k), udpT,
                            ALU.mult, ALU.max)
                        nc.vector.tensor_tensor(udc[:, :], m1[:, :], lc_diag,
                                                ALU.add)
                        break

                    # --- sweeps ---
                    rc_cur = RC0 if (k % 2) else RC1
                    tr_cur = reg(rc_cur, k, 0)
                    tc_cur = reg(rc_cur, k, 1)
                    tl_row = reg(LRC, k, 0)
                    tl_col = reg(LRC, k, 1)

                    # injects. D1: d0 = udp + BIG (prev diag from the prev
                    # row scan's D2 slot), d1 = -BIG. D2: d0 = m1*(k-1)/k
                    # (ACT scaled copy), d1 = lc_diag (rides the evictions).
                    if k == 1:
                        nc.gpsimd.memset(both(rc_cur, k)[:, :, :, 0], BIG)
                    else:
                        udp_sl = reg(rc_prev, k - 1, 0)[:, 1, :]
                        nc.scalar.activation(tr_cur[:, 0, :], udp_sl,
                                             ACTF.Copy, bias=BIG)
                        nc.scalar.activation(tc_cur[:, 0, :], udp_sl,
                                             ACTF.Copy, bias=BIG)
                        nc.scalar.activation(tr_cur[:, 1, :], m1[:, :],
                                             ACTF.Copy,
                                             scale=float((k - 1) / k))
                        nc.scalar.activation(tc_cur[:, 1, :], m1[:, :],
                                             ACTF.Copy,
                                             scale=float((k - 1) / k))
                    nc.gpsimd.memset(both(LRC, k)[:, :, :, 0], -BIG)
                    # col region's D2 d1 = lc_diag (row region's comes with
                    # the row chunk eviction below)
                    nc.scalar.activation(tl_col[:, 1, :], lc_diag, ACTF.Copy)

                    # evictions into d1 slots: b-frame plane p lands at
                    # slot p-k+2 (the diag plane p=k-1 lands at D2=slot 1).
                    for (c0, w, psv) in row_chunks:
                        nc.scalar.activation(
                            tl_row[:, 1 + c0:1 + c0 + w, :],
                            psv[:, 0:w * F].rearrange("p (n f) -> p n f", f=F),
                            ACTF.Copy)
                    for (c0, w, psv) in col_chunks:
                        nc.scalar.activation(
                            tl_col[:, 2 + c0:2 + c0 + w, :],
                            psv[:, 0:w * F].rearrange("p (n f) -> p n f", f=F),
                            ACTF.Copy)

                    # prep t data slots (data at slots 2..S-1)
                    if k == 1:
                        # D2 + data slots = 0 (disjoint from the D1 writes)
                        nc.gpsimd.memset(both(rc_cur, k)[:, :, :, 1:S], 0.0)
                        for w in range(2):
                            nc.vector.tensor_tensor_scan(
                                rc_cur[:, w * S * F:(w + 1) * S * F],
                                rc_cur[:, w * S * F:(w + 1) * S * F],
                                LRC[:, w * S * F:(w + 1) * S * F],
                                0.0, ALU.max, ALU.add)
                    else:
                        # t[j] = max(u_prev[j-1], u_prev[j] * (j-1)/j).
                        # (j-1)/j is CONSTANT per slot, so one per-slot
                        # scalar_tensor_tensor (imm scalar) fuses the mult
                        # and the max into a single streaming pass — half
                        # the element visits of the bulk mult+max pair.
                        # cur cell j at slot c=j-k+1; prev cell j' at slot
                        # j'-k+2 -> in0 = prev[c+1] (u_prev[j]),
                        # in1 = prev[c] (u_prev[j-1]).
                        # row cell j and col cell i at the same slot index
                        # share the same (j-1)/j factor -> one STT per slot
                        # covers BOTH regions (FD 512, one init instead of
                        # two). No cross-step barrier: both scans gate the
                        # next step anyway.
                        bc = both(rc_cur, k)
                        bp = both(rc_prev, k - 1)
                        for c in range(2, S):
                            j = k + c - 1
                            nc.vector.scalar_tensor_tensor(
                                bc[:, :, :, c], bp[:, :, :, c + 1],
                                float((j - 1) / j), bp[:, :, :, c],
                                ALU.mult, ALU.max)
                        for w in range(2):
                            nc.vector.tensor_tensor_scan(
                                rc_cur[:, w * S * F:(w + 1) * S * F],
                                rc_cur[:, w * S * F:(w + 1) * S * F],
                                LRC[:, w * S * F:(w + 1) * S * F],
                                0.0, ALU.max, ALU.add)

                    rc_prev = rc_cur

            out_sb = mp.tile([P, F], DT)
            nc.vector.tensor_scalar_mul(out_sb[:, :], ud[T % 2][:, :], 1.0 / T)
            nc.sync.dma_start(out_d.ap(), out_sb[:, :])

    nc.compile()
    return nc


def _consts():
    ident = np.eye(128, dtype=np.float32)
    return ident


def kernel(a: np.ndarray, b: np.ndarray) -> np.ndarray:
    a = np.ascontiguousarray(a, dtype=np.float32)
    b = np.ascontiguousarray(b, dtype=np.float32)
    assert a.shape == (NA, T, D) and b.shape == (NB, T, D)

    nc = build_program()
    ident = _consts()

    in_maps = []
    for core in range(8):
        ca, cb = core // 2, core % 2
        in_maps.append({
            "a_c": a[ca * ACH:(ca + 1) * ACH],
            "b_c": b[cb * BCH:(cb + 1) * BCH],
            "ident": ident,
        })

    res = bass_utils.run_bass_kernel_spmd(nc, in_maps, core_ids=list(range(8)))
    global _last_results
    _last_results = res

    out = np.zeros((NA, NB), dtype=np.float32)
    for core in range(8):
        ca, cb = core // 2, core % 2
        out[ca * ACH:(ca + 1) * ACH, cb * BCH:(cb + 1) * BCH] = \
            res.results[core]["out"]
    return out


# revision 24
# speedup vs baseline: 1.3741x; 1.0099x over previous
"""Trainium2 Bass kernel for nn_DynamicMaxSimilarity — scan-based dual-sweep DP.

Full inputs a,b: [512, 16, 256] f32.
  an = l2norm(tanh(a)) rows; bn likewise
  sim[a,b,i,j] = dot(an[a,i], bn[b,j]);  out[a,b] = DTW-like max-avg DP.

Sharding: 8 cores as 4 a-chunks (128) x 2 b-chunks (256). Per-core block
[128 a, 256 b]; pairs live as [128 partitions (a), 256 free (b)].

DP in the scaled domain u[i,j] = si[i,j]*max(i,j):
  step k: diag (k,k); row sweep cells (k, j>k); col sweep cells (i>k, k).
  Row sweep, per cell j: u = max(t[j], u_left) + lc, with
    t[j] = max(u_prev[j-1], u_prev[j]*(j-1)/j)   (prep, bulk)
  done by ONE tensor_tensor_scan (op0=max, op1=add) over free axis
  (pair-major, slots [inject, data...]): the inject slot (d0=diag+BIG,
  d1=-BIG) resets and seeds the per-pair chain. Col sweep symmetric.
Validated exactly vs the reference recurrence in fp64.
"""

import numpy as np

import concourse.bass as bass
from concourse import bacc
import concourse.mybir as mybir
from concourse.tile import TileContext
from concourse import bass_utils

NA, NB, T, D = 512, 512, 16, 256
ACH, BCH = 128, 256
P = 128
F = BCH              # pairs per partition
KH = D // 128
DT = mybir.dt.float32
F32R = mybir.dt.float32r
ALU = mybir.AluOpType
ACTF = mybir.ActivationFunctionType
BIG = 64.0

_last_results = None


def _normalize_block(nc, pool, wp, x_sb, nt, name):
    # per-quarter pipeline: tanh/sumsq chunks overlap the DMA tail and
    # each other across ACT/DVE
    ssq = pool.tile([P, nt], DT, name=f"{name}_ssq")
    nrm = pool.tile([P, nt], DT, name=f"{name}_nrm")
    rinv = pool.tile([P, nt], DT, name=f"{name}_rinv")
    h = nt // 4
    for q in range(4):
        sl = slice(q * h, (q + 1) * h)
        nc.scalar.activation(x_sb[:, sl, :], x_sb[:, sl, :], ACTF.Tanh)
        sq = wp.tile([P, h, D], DT, name=f"{name}_sq{q}", tag="sq_scr")
        nc.vector.tensor_tensor(sq[:, :, :], x_sb[:, sl, :], x_sb[:, sl, :],
                                ALU.mult)
        nc.vector.tensor_reduce(ssq[:, sl], sq[:, :, :], mybir.AxisListType.X,
                                ALU.add)
        # per-quarter norm finish so early quarters' scales don't wait on
        # the last quarter's sumsq
        nc.scalar.activation(nrm[:, sl], ssq[:, sl], ACTF.Sqrt)
        nc.vector.reciprocal(rinv[:, sl], nrm[:, sl])
        for i in range(q * h, (q + 1) * h):
            nc.vector.tensor_scalar_mul(x_sb[:, i, :], x_sb[:, i, :],
                                        rinv[:, i:i + 1])
    return x_sb


def build_program():
    nc = bacc.Bacc("TRN2", target_bir_lowering=False, debug=False)

    a_d = nc.dram_tensor("a_c", [ACH, T, D], DT, kind="ExternalInput")
    b_d = nc.dram_tensor("b_c", [BCH, T, D], DT, kind="ExternalInput")
    ident_d = nc.dram_tensor("ident", [128, 128], DT, kind="ExternalInput")
    out_d = nc.dram_tensor("out", [ACH, BCH], DT, kind="ExternalOutput")

    with TileContext(nc) as tc:
        with (
            tc.tile_pool(name="mp", bufs=1) as mp,
            tc.tile_pool(name="wp", bufs=2) as wp,
        ):
            ident = mp.tile([128, 128], DT)
            nc.sync.dma_start(ident[:, :], ident_d.ap())

            # ---- load + normalize (in place) ----
            a_sb = mp.tile([P, T, D], DT, tag="sh_lrow")
            for q in range(4):
                nc.sync.dma_start(a_sb[:, q * 4:(q + 1) * 4, :],
                                  a_d.ap()[:, q * 4:(q + 1) * 4, :])
            an = _normalize_block(nc, mp, wp, a_sb, T, "a")

            b_sb = [mp.tile([P, T, D], DT, name=f"b_sb{h}", tag=f"sh_b{h}")
                    for h in range(2)]
            for h in range(2):
                for q in range(4):
                    nc.sync.dma_start(
                        b_sb[h][:, q * 4:(q + 1) * 4, :],
                        b_d.ap()[h * 128:(h + 1) * 128, q * 4:(q + 1) * 4, :])
            bn = [_normalize_block(nc, mp, wp, b_sb[h], T, f"b{h}")
                  for h in range(2)]

            # ---- transposes into matmul layout ----
            anT = mp.tile([P, T, 2, 128], F32R)
            bnT = [mp.tile([P, T, BCH], F32R, name=f"bnT{kh}") for kh in range(KH)]
            with tc.tile_pool(name="tpp", bufs=4, space="PSUM") as tpp:
                def tp_a_group(i0):
                    # 4 transposes (2 i x 2 kh) per PSUM bank, 1 evict
                    ps = tpp.tile([128, 512], DT, name=f"tpa{i0}", tag="tpa",
                                  bufs=2)
                    for q, (i, kh) in enumerate(
                            (i0 + di, kh) for di in range(2) for kh in range(KH)):
                        nc.tensor.transpose(ps[:, q * 128:(q + 1) * 128],
                                            an[:, i, kh * 128:(kh + 1) * 128],
                                            ident[:, :])
                    nc.scalar.activation(
                        anT[:, i0:i0 + 2, :, :],
                        ps[:, :].rearrange("p (i k a) -> p i k a", i=2, k=2),
                        ACTF.Copy)

                # frame 0+1 a-group first (gates step 1's row piece), then
                # all b transposes, then the remaining a-groups (only gate
                # the later col pieces).
                tp_a_group(0)
                for j in range(T):
                    for kh in range(KH):
                        ps = tpp.tile([128, 256], DT, name=f"tpb{j}{kh}",
                                      tag="tpb", bufs=6)
                        for bh in range(2):
                            nc.tensor.transpose(
                                ps[:, bh * 128:(bh + 1) * 128],
                                bn[bh][:, j, kh * 128:(kh + 1) * 128],
                                ident[:, :])
                        if j % 2 == 0:
                            nc.scalar.activation(bnT[kh][:, j, :], ps[:, :],
                                                 ACTF.Copy)
                        else:
                            nc.vector.tensor_copy(bnT[kh][:, j, :], ps[:, :])
                for i0 in range(2, T, 2):
                    tp_a_group(i0)

            # ---- DP tiles ----
            # Row and col sweep state are PACKED in one buffer per step:
            # [row region | col region], each S*F (S=17-k, per-pair slots
            # [inject, data...]), scanned per region (a merged scan acts as
            # a barrier and costs ~5us). a_sb/b_sb memory reused by tags.
            # per-pair slots: [D1, D2, data j=k+1..16]; Sl = 18-k.
            # D1 (d0=udp+BIG, d1=-BIG) resets state to the previous diag;
            # D2 (d0=max(u[k-1,k],u[k,k-1])*(k-1)/k, d1=lc[k,k]) computes the
            # step-k diag INSIDE the scan; its output slot is next step's udp.
            RC0 = mp.tile([P, 2 * 18 * F], DT, tag="sh_lrow")
            RC1 = mp.tile([P, 2 * 18 * F], DT, tag="sh_b0")
            LRC = mp.tile([P, 2 * 18 * F], DT, tag="sh_b1")
            ud = [mp.tile([P, F], DT, name=f"ud{x}") for x in range(2)]

            def reg(tile, k, w):
                """Region w (0=row, 1=col) as [P, slots, pairs] (s, f)."""
                S = 18 - k
                return tile[:, w * S * F:(w + 1) * S * F].rearrange(
                    "p (f s) -> p s f", s=S)

            def reg_fs(tile, k, w):
                """Region w as [P, pairs, slots] (slots contiguous)."""
                S = 18 - k
                return tile[:, w * S * F:(w + 1) * S * F].rearrange(
                    "p (f s) -> p f s", s=S)

            def both(tile, k):
                """[P, region, pairs, slots] 4D view over both regions."""
                S = 18 - k
                return tile[:, 0:2 * S * F].rearrange(
                    "p (w f s) -> p w f s", w=2, s=S)

            rc_prev = None

            # ---- fused matmul + dual-sweep DP ----
            with tc.tile_pool(name="pp", bufs=2, space="PSUM") as pp:
                def mm_planes(psv, i0, j0, np_):
                    """matmul lc planes (frame i0; b-frames j0..j0+np_) into
                    psv [128, np_*F]."""
                    for q0 in range(0, np_, 2):
                        w = min(2, np_ - q0)
                        for kh in range(KH):
                            nc.tensor.matmul(
                                psv[:, q0 * F:(q0 + w) * F],
                                anT[:, i0, kh, :],
                                bnT[kh][:, j0 + q0:j0 + q0 + w, :],
                                start=(kh == 0), stop=(kh == KH - 1),
                            )

                for k in range(1, T + 1):
                    S = 18 - k   # slots per pair: [D1, D2, data...]
                    udp, udc = ud[(k - 1) % 2], ud[k % 2]

                    # --- produce lc L-border for step k ---
                    # diag+row piece: frame k-1 x b-frames k-1..15
                    nrow = 17 - k   # includes diag plane
                    row_chunks = []
                    for c0 in range(0, nrow, 2):
                        w = min(2, nrow - c0)
                        psv = pp.tile([128, 2 * F], DT,
                                      name=f"psr{k}_{c0}", tag="locr", bufs=4)
                        mm_planes(psv, k - 1, (k - 1) + c0, w)
                        row_chunks.append((c0, w, psv))
                    # col piece: frames k..15 x b-frame k-1
                    ncol = 16 - k
                    col_chunks = []
                    for c0 in range(0, ncol, 2):
                        w = min(2, ncol - c0)
                        psv = pp.tile([128, 2 * F], DT,
                                      name=f"psc{k}_{c0}", tag="locc", bufs=4)
                        for q in range(w):
                            for kh in range(KH):
                                nc.tensor.matmul(
                                    psv[:, q * F:(q + 1) * F],
                                    anT[:, k + c0 + q, kh, :],
                                    bnT[kh][:, k - 1:k, :],
                                    start=(kh == 0), stop=(kh == KH - 1),
                                )
                        col_chunks.append((c0, w, psv))

                    # --- diag cell (k,k) ---
                    lc_diag = row_chunks[0][2][:, 0:F]
                    m1 = None
                    if k > 1:
                        m1 = wp.tile([P, F], DT, name=f"m1_{k}", tag="m1")
                        nc.vector.tensor_tensor(
                            m1[:, :], reg(rc_prev, k - 1, 0)[:, 2, :],
                            reg(rc_prev, k - 1, 1)[:, 2, :], ALU.max)
                    if k == T:
                        # no scans at k=16: explicit diag
                        udpT = reg(rc_prev, k - 1, 0)[:, 1, :]
                        nc.vector.scalar_tensor_tensor(
                            m1[:, :], m1[:, :], float((k - 1) / k), udpT,
                            ALU.mult, ALU.max)
                        nc.vector.tensor_tensor(udc[:, :], m1[:, :], lc_diag,
                                                ALU.add)
                        break

                    # --- sweeps ---
                    rc_cur = RC0 if (k % 2) else RC1
                    tr_cur = reg(rc_cur, k, 0)
                    tc_cur = reg(rc_cur, k, 1)
                    tl_row = reg(LRC, k, 0)
                    tl_col = reg(LRC, k, 1)

                    # injects. D1: d0 = udp + BIG (prev diag from the prev
                    # row scan's D2 slot), d1 = -BIG. D2: d0 = m1*(k-1)/k
                    # (ACT scaled copy), d1 = lc_diag (rides the evictions).
                    if k == 1:
                        nc.gpsimd.memset(both(rc_cur, k)[:, :, :, 0], BIG)
                    else:
                        udp_sl = reg(rc_prev, k - 1, 0)[:, 1, :]
                        nc.scalar.activation(tr_cur[:, 0, :], udp_sl,
                                             ACTF.Copy, bias=BIG)
                        nc.scalar.activation(tc_cur[:, 0, :], udp_sl,
                                             ACTF.Copy, bias=BIG)
                        nc.scalar.activation(tr_cur[:, 1, :], m1[:, :],
                                             ACTF.Copy,
                                             scale=float((k - 1) / k))
                        nc.scalar.activation(tc_cur[:, 1, :], m1[:, :],
                                             ACTF.Copy,
                                             scale=float((k - 1) / k))
                    nc.gpsimd.memset(both(LRC, k)[:, :, :, 0], -BIG)
                    # col region's D2 d1 = lc_diag (row region's comes with
                    # the row chunk eviction below)
                    nc.scalar.activation(tl_col[:, 1, :], lc_diag, ACTF.Copy)

                    # evictions into d1 slots: b-frame plane p lands at
                    # slot p-k+2 (the diag plane p=k-1 lands at D2=slot 1).
                    for (c0, w, psv) in row_chunks:
                        nc.scalar.activation(
                            tl_row[:, 1 + c0:1 + c0 + w, :],
                            psv[:, 0:w * F].rearrange("p (n f) -> p n f", f=F),
                            ACTF.Copy)
                    for (c0, w, psv) in col_chunks:
                        nc.scalar.activation(
                            tl_col[:, 2 + c0:2 + c0 + w, :],
                            psv[:, 0:w * F].rearrange("p (n f) -> p n f", f=F),
                            ACTF.Copy)

                    # prep t data slots (data at slots 2..S-1)
                    if k == 1:
                        # D2 + data slots = 0 (disjoint from the D1 writes)
                        nc.gpsimd.memset(both(rc_cur, k)[:, :, :, 1:S], 0.0)
                        for w in range(2):
                            nc.vector.tensor_tensor_scan(
                                rc_cur[:, w * S * F:(w + 1) * S * F],
                                rc_cur[:, w * S * F:(w + 1) * S * F],
                                LRC[:, w * S * F:(w + 1) * S * F],
                                0.0, ALU.max, ALU.add)
                    else:
                        # t[j] = max(u_prev[j-1], u_prev[j] * (j-1)/j).
                        # (j-1)/j is CONSTANT per slot, so one per-slot
                        # scalar_tensor_tensor (imm scalar) fuses the mult
                        # and the max into a single streaming pass — half
                        # the element visits of the bulk mult+max pair.
                        # cur cell j at slot c=j-k+1; prev cell j' at slot
                        # j'-k+2 -> in0 = prev[c+1] (u_prev[j]),
                        # in1 = prev[c] (u_prev[j-1]).
                        # row cell j and col cell i at the same slot index
                        # share the same (j-1)/j factor -> one STT per slot
                        # covers BOTH regions (FD 512, one init instead of
                        # two). No cross-step barrier: both scans gate the
                        # next step anyway.
                        bc = both(rc_cur, k)
                        bp = both(rc_prev, k - 1)
                        for c in range(2, S):
                            j = k + c - 1
                            nc.vector.scalar_tensor_tensor(
                                bc[:, :, :, c], bp[:, :, :, c + 1],
                                float((j - 1) / j), bp[:, :, :, c],
                                ALU.mult, ALU.max)
                        for w in range(2):
                            nc.vector.tensor_tensor_scan(
                                rc_cur[:, w * S * F:(w + 1) * S * F],
                                rc_cur[:, w * S * F:(w + 1) * S * F],
                                LRC[:, w * S * F:(w + 1) * S * F],
                                0.0, ALU.max, ALU.add)

                    rc_prev = rc_cur

            out_sb = mp.tile([P, F], DT)
            nc.vector.tensor_scalar_mul(out_sb[:, :], ud[T % 2][:, :], 1.0 / T)
            nc.sync.dma_start(out_d.ap(), out_sb[:, :])

    nc.compile()
    return nc


def _consts():
    ident = np.eye(128, dtype=np.float32)
    return ident


def kernel(a: np.ndarray, b: np.ndarray) -> np.ndarray:
    a = np.ascontiguousarray(a, dtype=np.float32)
    b = np.ascontiguousarray(b, dtype=np.float32)
    assert a.shape == (NA, T, D) and b.shape == (NB, T, D)

    nc = build_program()
    ident = _consts()

    in_maps = []
    for core in range(8):
        ca, cb = core // 2, core % 2
        in_maps.append({
            "a_c": a[ca * ACH:(ca + 1) * ACH],
            "b_c": b[cb * BCH:(cb + 1) * BCH],
            "ident": ident,
        })

    res = bass_utils.run_bass_kernel_spmd(nc, in_maps, core_ids=list(range(8)))
    global _last_results
    _last_results = res

    out = np.zeros((NA, NB), dtype=np.float32)
    for core in range(8):
        ca, cb = core // 2, core % 2
        out[ca * ACH:(ca + 1) * ACH, cb * BCH:(cb + 1) * BCH] = \
            res.results[core]["out"]
    return out
